# revision 1
# baseline (speedup 1.0000x reference)
"""Trainium2 Bass kernel for nn_LogicConvSparseMatrix.

Math: the reference's 15-term weighted logic-op sum collapses to

    out[b,k] = C_ab[k]*A*B + C_a[k]*A + C_b[k]*B + C_1[k]

where A = x[b, ca_k, ha_k+oh, wa_k+ow], B = x[b, cb_k, hb_k+oh, wb_k+ow]
are shifted 126x126 windows.  With alpha = C_b/C_ab, gamma = C_1 -
C_a*C_b/C_ab this factors into

    out = (A + alpha) * (C_ab*B + C_a) + gamma

Per kernel k (three element passes; two ops cannot carry 4 coefficients):
  1. ScalarE affine:  B2 = C_ab*B + C_a
  2. VectorE scalar_tensor_tensor:  T = (A + alpha) * B2
  3. "+gamma", load-balanced per group of 8 k's across:
       - ScalarE Copy(T*1 + gamma) in place,
       - VectorE tensor_scalar add (AP shaped [4,63] to force 1x mode so
         it never grabs the DVE/GpSimd shared SBUF port), or
       - GpSimd tensor_tensor T + gcol (broadcast gamma table; GpSimd's
         tensor_scalar kernel is pathologically slow, tensor_tensor is ok).

Index pairs are known at build time, so gathers are compile-time SBUF
views of X[p=h, (c,b,w)].  Compute-engine SBUF operands may only start
at partition 0/32/64/96; the relative h-shift between the two windows is
materialized as shifted column copies via SBUF->SBUF DMA (DMA may
address any partition), consolidated into gap-bridged contiguous
channel-range runs (one DMA each).  All compute APs start at partition
0; store DMAs select rows [base : base+126].

k's are processed sorted by base so stores batch into ~1MB run DMAs
issued from the (otherwise idle) GpSimd queue via SWDGE, whose issue
cost is ~0.7us and whose transfers run async; HWDGE queue transfers
block their issuing engine, so loads/shift-copies are split between the
SP queue (batch 0 + copies) and Activation queue (batch 1, issued while
ScalarE is still idle).  The device output layout is [K, BPC, OH, OW]
with k's in base-sorted order; the host inverse-permutes/transposes.
Sharding: data-parallel over batch, 2 batch items per core, 8 cores.
"""

import numpy as np

B, C, H, W = 16, 64, 128, 128
K = 128
RH = RW = 3
OH, OW = H - RH + 1, W - RW + 1
NCORES = 8
BPC = B // NCORES

GRP = 8  # kernels per store group
GSPLIT = ("gp", "gp", "dve", "act")  # gamma-engine per group, round-robin


def _coeffs(weights):
    """Per-kernel coefficients of out = Cab*a*b + Ca*a + Cb*b + C1."""
    w = [weights[:, i].astype(np.float64) for i in range(16)]
    cab = w[1] - w[2] - w[4] - 2 * w[6] - w[7] + w[8] + 2 * w[9] + w[11] + w[13] - w[14]
    ca = w[2] + w[3] + w[6] + w[7] - w[8] - w[9] - w[12] - w[13]
    cb = w[4] + w[5] + w[6] + w[7] - w[8] - w[9] - w[10] - w[11]
    c1 = w[8] + w[9] + w[10] + w[11] + w[12] + w[13] + w[14] + w[15]
    return cab, ca, cb, c1


def _plan(pairs_a, pairs_b, weights):
    """Host-side schedule.  Returns (plans, runs, order, gcol) where
    plans[k] = (k, base, a_src, b_src, path, scal, gamma) with
    a_src/b_src = (from_shifted, column_index, w_off), runs = list of
    (shift, c0, c1, dest_col0) shifted-copy DMAs plus total column count,
    order = base-sorted k order, gcol = broadcast gamma table."""
    cab, ca, cb, c1 = _coeffs(weights)
    keys = {}  # (shift, chan) -> use count; shift != 0
    raw = []
    for k in range(K):
        ha, wa, cca = int(pairs_a[k][0]), int(pairs_a[k][1]), int(pairs_a[k][2])
        hb, wb, ccb = int(pairs_b[k][0]), int(pairs_b[k][1]), int(pairs_b[k][2])
        if ha == hb:
            base = ha
            a_key, b_key = (0, cca), (0, ccb)
        else:
            # shifting either side keeps that copy's invalid rows inside the
            # junk-lane range (min_h + |delta| <= 2); reuse existing columns.
            if ha < hb:  # a is the smaller-h side
                neg = ((ha - hb, cca), True, hb)  # (col key, shifts_a, base)
                pos = ((hb - ha, ccb), False, ha)
            else:
                neg = ((hb - ha, ccb), False, ha)
                pos = ((ha - hb, cca), True, hb)
            key, shift_a, base = pos if (pos[0] in keys and neg[0] not in keys) else neg
            keys[key] = keys.get(key, 0) + 1
            if shift_a:
                a_key, b_key = key, (0, ccb)
            else:
                a_key, b_key = (0, cca), key

        kab, kka, kkb, kk1 = float(cab[k]), float(ca[k]), float(cb[k]), float(c1[k])
        if abs(kab) <= 1e-7:
            path, scal, gamma = "linear", (kka, kkb, kk1), 0.0
        elif abs(kkb) <= 50.0 * abs(kab) and abs(kka * kkb) <= 50.0 * abs(kab):
            path = "fact"
            scal = (kab, kka, kkb / kab)
            gamma = kk1 - kka * kkb / kab
        else:
            path, scal, gamma = "exact", (kab, kka, kkb, kk1), 0.0
        raw.append((k, base, a_key, wa, b_key, wb, path, scal, gamma))

    # consolidate shifted columns into gap-bridged contiguous c-runs
    def build_runs(gaptol):
        runs, cmap, total = [], {}, 0
        for s in sorted({sc[0] for sc in keys}):
            cs = sorted(c for (s2, c) in keys if s2 == s)
            i = 0
            while i < len(cs):
                j = i
                while j + 1 < len(cs) and cs[j + 1] - cs[j] <= gaptol:
                    j += 1
                c0, cl = cs[i], cs[j]
                for c in range(c0, cl + 1):
                    cmap[(s, c)] = total + (c - c0)
                runs.append((s, c0, cl, total))
                total += cl - c0 + 1
                i = j + 1
        return runs, cmap, total

    for gaptol in (8, 4, 1, 0):
        runlist, cmap, ncols = build_runs(gaptol)
        if ncols <= 75:
            break

    plans = []
    for (k, base, a_key, wa, b_key, wb, path, scal, gamma) in raw:
        a_src = (False, a_key[1], wa) if a_key[0] == 0 else (True, cmap[a_key], wa)
        b_src = (False, b_key[1], wb) if b_key[0] == 0 else (True, cmap[b_key], wb)
        plans.append((k, base, a_src, b_src, path, scal, gamma))

    order = sorted(
        range(K), key=lambda k: (plans[k][1], plans[k][2][0] or plans[k][3][0], k)
    )  # by base, no-shift kernels first within each base run
    gcol = np.zeros((H, K), np.float32)
    for pos, k in enumerate(order):
        gcol[:, pos] = plans[k][6]
    return plans, (runlist, ncols), order, gcol


def _build(pairs_a, pairs_b, weights):
    import concourse.bacc as bacc
    import concourse.mybir as mybir
    from concourse.tile import TileContext

    f32 = mybir.dt.float32
    Copy = mybir.ActivationFunctionType.Copy
    add, mult = mybir.AluOpType.add, mybir.AluOpType.mult

    plans, (runlist, ncols), order, gcol_np = _plan(pairs_a, pairs_b, weights)
    ncols = max(1, ncols)
    ngrp = (K + GRP - 1) // GRP

    if ncols > 80:
        raise RuntimeError(f"shifted-column budget exceeded: {ncols}")

    nc = bacc.Bacc()
    x = nc.dram_tensor("x", [C, H, BPC, W], f32, kind="ExternalInput")
    gcd = nc.dram_tensor("gcol", [H, K], f32, kind="ExternalInput")
    out = nc.dram_tensor("out", [K, BPC, OH, OW], f32, kind="ExternalOutput")

    with TileContext(nc) as tc:
        with (
            tc.tile_pool(name="xp", bufs=1) as xp,
            tc.tile_pool(name="bp", bufs=6) as bp,
            tc.tile_pool(name="tp", bufs=3) as tp,
            tc.tile_pool(name="op", bufs=2) as op,
        ):
            # x arrives host-transposed as [C, H, BPC, W] so both the main
            # staging load and the shifted-run loads are straight 3-dim
            # DRAM->SBUF DMAs (shifted SBUF->SBUF copies measured ~40 GB/s).
            xr = x.rearrange("c h b w -> h c (b w)")
            X = xp.tile([H, C * BPC * W], f32)
            Xv = X.rearrange("p (c b w) -> p c b w", c=C, b=BPC)
            Xf = X.rearrange("p (c q) -> p c q", c=C)
            half = C // 2
            nc.sync.dma_start(out=Xf[:, 0:half], in_=xr[:, 0:half])
            nc.sync.dma_start(out=Xf[:, half:C], in_=xr[:, half:C])

            S = xp.tile([H, ncols * BPC * W], f32)
            Sv = S.rearrange("p (j b w) -> p j b w", j=ncols, b=BPC)
            Sf = S.rearrange("p (j q) -> p j q", j=ncols)
            # finite filler for shifted-run head/tail rows (junk lanes only)
            for d0 in range(0, ncols, C):
                n = min(C, ncols - d0)
                nc.sync.dma_start(out=Sf[0:2, d0 : d0 + n], in_=xr[0:2, 0:n])
                nc.sync.dma_start(out=Sf[H - 2 : H, d0 : d0 + n], in_=xr[0:2, 0:n])
            for ri, (s, c0, cl, d0) in enumerate(runlist):
                # S[p, d0+i] = x[c0+i, p+s], loaded from DRAM.  All loads stay
                # on the SP queue: a compute engine's stream blocks on its own
                # queue's transfers, so Activation must carry no DMAs.
                eng = nc.sync
                n = cl - c0 + 1
                if s < 0:
                    eng.dma_start(
                        out=Sf[-s:H, d0 : d0 + n], in_=xr[0 : H + s, c0 : c0 + n]
                    )
                else:
                    eng.dma_start(
                        out=Sf[0 : H - s, d0 : d0 + n], in_=xr[s:H, c0 : c0 + n]
                    )

            Gc = xp.tile([H, K], f32)
            nc.sync.dma_start(out=Gc, in_=gcd[:, :])

            out_kb = out.rearrange("k b oh ow -> (k b) oh ow")
            fd = BPC * OW

            def emit_gamma_and_store(g, ks, geng, T, O):
                # deferred one group so cross-engine waits are pre-satisfied
                for j, k in enumerate(ks):
                    _, base, _, _, path, scal, gamma = plans[k]
                    cnt = base + OH
                    slot = T[0:cnt, j * fd : (j + 1) * fd]
                    if gamma != 0.0 or geng == "gp":
                        pos = g * GRP + j
                        if geng == "act":
                            nc.scalar.activation(
                                slot, slot, Copy, bias=gamma, scale=1.0
                            )
                        elif geng == "dve":
                            # odd innermost dim forces 1x mode: no shared-port
                            # contention with GpSimd
                            so = slot.rearrange("p (a q) -> p a q", a=4)
                            nc.vector.tensor_scalar(so, so, gamma, None, add)
                        else:
                            gb = Gc[0:cnt, pos : pos + 1].broadcast_to([cnt, fd])
                            osl = O[0:cnt, j * fd : (j + 1) * fd]
                            nc.gpsimd.tensor_tensor(osl, slot, gb, add)
                # batched stores per same-base run: SWDGE on the GpSimd queue
                # (issue ~0.7us, transfer async; HWDGE would block its engine).
                src_t = O if geng == "gp" else T
                i = 0
                while i < len(ks):
                    base = plans[ks[i]][1]
                    i2 = i
                    while i2 < len(ks) and plans[ks[i2]][1] == base:
                        i2 += 1
                    src = src_t[base : base + OH, i * fd : i2 * fd].rearrange(
                        "p (kb w) -> p kb w", w=OW
                    )
                    dst = out_kb[(g * GRP + i) * BPC : (g * GRP + i2) * BPC]
                    nc.gpsimd.dma_start(
                        out=dst.rearrange("kb oh ow -> oh kb ow"), in_=src
                    )
                    i = i2

            pending = None
            for g in range(ngrp):
                ks = order[g * GRP : (g + 1) * GRP]
                geng = GSPLIT[g % len(GSPLIT)]
                T = tp.tile([H, GRP * fd], f32, tag="t", name=f"t_{g}")
                O = None
                if geng == "gp":
                    O = op.tile([H, GRP * fd], f32, tag="o", name=f"o_{g}")

                for j, k in enumerate(ks):
                    _, base, a_src, b_src, path, scal, gamma = plans[k]
                    cnt = base + OH

                    def view(src):
                        shifted, idx, woff = src
                        t = Sv if shifted else Xv
                        return t[0:cnt, idx, :, woff : woff + OW]

                    Av, Bv = view(a_src), view(b_src)
                    slot = T[0:cnt, j * fd : (j + 1) * fd]
                    slotv = slot.rearrange("p (b w) -> p b w", b=BPC)
                    b2 = bp.tile([H, fd], f32, tag="b2", name=f"b2_{k}")
                    b2v = b2.rearrange("p (b w) -> p b w", b=BPC)[0:cnt]

                    if path == "fact":
                        kab, kka, alpha = scal
                        nc.scalar.activation(b2v, Bv, Copy, bias=kka, scale=kab)
                        nc.vector.scalar_tensor_tensor(slotv, Av, alpha, b2v, add, mult)
                    else:  # linear/exact: slot = Ca*A + (Cb*B + C1)
                        if path == "linear":
                            kka, kkb, kk1 = scal
                        else:
                            kab, kka, kkb, kk1 = scal
                        nc.scalar.activation(b2v, Bv, Copy, bias=kk1, scale=kkb)
                        nc.vector.scalar_tensor_tensor(slotv, Av, kka, b2v, mult, add)
                        if path == "exact":  # += (Cab*B)*A
                            p2 = bp.tile([H, fd], f32, tag="b2", name=f"p2_{k}")
                            p2v = p2.rearrange("p (b w) -> p b w", b=BPC)[0:cnt]
                            nc.vector.scalar_tensor_tensor(p2v, Bv, kab, Av, mult, mult)
                            nc.vector.tensor_tensor(slot, slot, p2[0:cnt], add)

                if pending is not None:
                    emit_gamma_and_store(*pending)
                pending = (g, ks, geng, T, O)
            if pending is not None:
                emit_gamma_and_store(*pending)
    nc.compile()
    return nc


def _consts(pairs_a, pairs_b, weights):
    plans, runs, order, gcol = _plan(pairs_a, pairs_b, weights)
    return {"gcol": gcol}, order


def kernel(x, pairs_a, pairs_b, weights):
    from concourse.bass_utils import run_bass_kernel_spmd

    x = np.ascontiguousarray(np.asarray(x), dtype=np.float32)
    pa = np.asarray(pairs_a).astype(np.int64)
    pb = np.asarray(pairs_b).astype(np.int64)
    w = np.asarray(weights).astype(np.float32)

    nc = _build(pa, pb, w)
    extra, order = _consts(pa, pb, w)
    in_maps = [
        {
            "x": np.ascontiguousarray(
                x[i * BPC : (i + 1) * BPC].transpose(1, 2, 0, 3)
            ),
            **extra,
        }
        for i in range(NCORES)
    ]
    res = run_bass_kernel_spmd(nc, in_maps, core_ids=list(range(NCORES)))
    # device layout [K(sorted), BPC, OH, OW] per core -> [B, K, OH, OW]
    full = np.concatenate([r["out"] for r in res.results], axis=1)  # [K, B, ...]
    pos = np.empty(K, np.int64)
    pos[np.asarray(order)] = np.arange(K)
    return np.ascontiguousarray(full[pos].transpose(1, 0, 2, 3))



# revision 2
# speedup vs baseline: 1.1260x; 1.1260x over previous
"""Trainium2 Bass kernel for nn_LogicConvSparseMatrix.

Math: the reference's 15-term weighted logic-op sum collapses to

    out[b,k] = C_ab[k]*A*B + C_a[k]*A + C_b[k]*B + C_1[k]

where A = x[b, ca_k, ha_k+oh, wa_k+ow], B = x[b, cb_k, hb_k+oh, wb_k+ow]
are shifted 126x126 windows.  With alpha = C_b/C_ab, gamma = C_1 -
C_a*C_b/C_ab this factors into

    out = (A + alpha) * (C_ab*B + C_a) + gamma

Per kernel k (three element passes; two ops cannot carry 4 coefficients):
  1. ScalarE affine:  B2 = C_ab*B + C_a
  2. VectorE scalar_tensor_tensor:  T = (A + alpha) * B2
  3. "+gamma", load-balanced per group of 8 k's across ScalarE / VectorE /
     GpSimd.

Index pairs are known at build time, so gathers are compile-time SBUF
views of X[p=h, (c,b,w)].  Compute-engine SBUF operands may only start
at partition 0/32/64/96; the relative h-shift between the two windows is
materialized as shifted column copies loaded straight from DRAM,
consolidated into gap-bridged contiguous channel-range runs.

DMA layout (the v1 bottleneck was 1KB load descriptors skewed onto one
SDMA engine): x arrives host-transposed as [H, C, BPC, W] so every load
descriptor is a per-partition CONTIGUOUS run (64KB for the main staging
load, n KB for shifted runs).  The device output layout is
[OH, K(sorted), BPC, OW] so each group store is one ~8KB contiguous run
per partition (~1MB per store DMA).  Stores issue from the GpSimd queue
via SWDGE (async); loads from the Sync queue via HWDGE.
Sharding: data-parallel over batch, 2 batch items per core, 8 cores.
"""

import numpy as np

B, C, H, W = 16, 64, 128, 128
K = 128
RH = RW = 3
OH, OW = H - RH + 1, W - RW + 1
NCORES = 8
BPC = B // NCORES

GRP = 8  # kernels per store group
GSPLIT = ("gp", "gp", "dve", "act")  # gamma-engine per group, round-robin


def _coeffs(weights):
    """Per-kernel coefficients of out = Cab*a*b + Ca*a + Cb*b + C1."""
    w = [weights[:, i].astype(np.float64) for i in range(16)]
    cab = w[1] - w[2] - w[4] - 2 * w[6] - w[7] + w[8] + 2 * w[9] + w[11] + w[13] - w[14]
    ca = w[2] + w[3] + w[6] + w[7] - w[8] - w[9] - w[12] - w[13]
    cb = w[4] + w[5] + w[6] + w[7] - w[8] - w[9] - w[10] - w[11]
    c1 = w[8] + w[9] + w[10] + w[11] + w[12] + w[13] + w[14] + w[15]
    return cab, ca, cb, c1


def _plan(pairs_a, pairs_b, weights):
    """Host-side schedule.  Returns (plans, runs, order, gcol) where
    plans[k] = (k, base, a_src, b_src, path, scal, gamma) with
    a_src/b_src = (from_shifted, column_index, w_off), runs = list of
    (shift, c0, c1, dest_col0) shifted-copy DMAs plus total column count,
    order = base-sorted k order, gcol = broadcast gamma table."""
    cab, ca, cb, c1 = _coeffs(weights)
    keys = {}  # (shift, chan) -> use count; shift != 0
    raw = []
    for k in range(K):
        ha, wa, cca = int(pairs_a[k][0]), int(pairs_a[k][1]), int(pairs_a[k][2])
        hb, wb, ccb = int(pairs_b[k][0]), int(pairs_b[k][1]), int(pairs_b[k][2])
        if ha == hb:
            base = ha
            a_key, b_key = (0, cca), (0, ccb)
        else:
            # shifting either side keeps that copy's invalid rows inside the
            # junk-lane range (min_h + |delta| <= 2); reuse existing columns.
            if ha < hb:  # a is the smaller-h side
                neg = ((ha - hb, cca), True, hb)  # (col key, shifts_a, base)
                pos = ((hb - ha, ccb), False, ha)
            else:
                neg = ((hb - ha, ccb), False, ha)
                pos = ((ha - hb, cca), True, hb)
            key, shift_a, base = pos if (pos[0] in keys and neg[0] not in keys) else neg
            keys[key] = keys.get(key, 0) + 1
            if shift_a:
                a_key, b_key = key, (0, ccb)
            else:
                a_key, b_key = (0, cca), key

        kab, kka, kkb, kk1 = float(cab[k]), float(ca[k]), float(cb[k]), float(c1[k])
        if abs(kab) <= 1e-7:
            path, scal, gamma = "linear", (kka, kkb, kk1), 0.0
        elif abs(kkb) <= 50.0 * abs(kab) and abs(kka * kkb) <= 50.0 * abs(kab):
            path = "fact"
            scal = (kab, kka, kkb / kab)
            gamma = kk1 - kka * kkb / kab
        else:
            path, scal, gamma = "exact", (kab, kka, kkb, kk1), 0.0
        raw.append((k, base, a_key, wa, b_key, wb, path, scal, gamma))

    # consolidate shifted columns into gap-bridged contiguous c-runs
    def build_runs(gaptol):
        runs, cmap, total = [], {}, 0
        for s in sorted({sc[0] for sc in keys}):
            cs = sorted(c for (s2, c) in keys if s2 == s)
            i = 0
            while i < len(cs):
                j = i
                while j + 1 < len(cs) and cs[j + 1] - cs[j] <= gaptol:
                    j += 1
                c0, cl = cs[i], cs[j]
                for c in range(c0, cl + 1):
                    cmap[(s, c)] = total + (c - c0)
                runs.append((s, c0, cl, total))
                total += cl - c0 + 1
                i = j + 1
        return runs, cmap, total

    for gaptol in (8, 4, 1, 0):
        runlist, cmap, ncols = build_runs(gaptol)
        if ncols <= 75:
            break

    plans = []
    for (k, base, a_key, wa, b_key, wb, path, scal, gamma) in raw:
        a_src = (False, a_key[1], wa) if a_key[0] == 0 else (True, cmap[a_key], wa)
        b_src = (False, b_key[1], wb) if b_key[0] == 0 else (True, cmap[b_key], wb)
        plans.append((k, base, a_src, b_src, path, scal, gamma))

    order = sorted(
        range(K), key=lambda k: (plans[k][1], plans[k][2][0] or plans[k][3][0], k)
    )  # by base, no-shift kernels first within each base run
    gcol = np.zeros((H, K), np.float32)
    for pos, k in enumerate(order):
        gcol[:, pos] = plans[k][6]
    return plans, (runlist, ncols), order, gcol


def _build(pairs_a, pairs_b, weights):
    import concourse.bacc as bacc
    import concourse.mybir as mybir
    from concourse.tile import TileContext

    f32 = mybir.dt.float32
    Copy = mybir.ActivationFunctionType.Copy
    add, mult = mybir.AluOpType.add, mybir.AluOpType.mult

    plans, (runlist, ncols), order, gcol_np = _plan(pairs_a, pairs_b, weights)
    ncols = max(1, ncols)
    ngrp = (K + GRP - 1) // GRP

    if ncols > 80:
        raise RuntimeError(f"shifted-column budget exceeded: {ncols}")

    nc = bacc.Bacc()
    # x host-transposed to [H, C, BPC, W]: per-partition (h) loads are
    # contiguous DRAM runs -> few large DMA descriptors.
    x = nc.dram_tensor("x", [H, C, BPC, W], f32, kind="ExternalInput")
    gcd = nc.dram_tensor("gcol", [H, K], f32, kind="ExternalInput")
    # output [OH, K, BPC, OW]: a group store is one contiguous run per
    # partition (oh).  Host inverse-permutes k and transposes.
    out = nc.dram_tensor("out", [OH, K, BPC, OW], f32, kind="ExternalOutput")

    with TileContext(nc) as tc:
        with (
            tc.tile_pool(name="xp", bufs=1) as xp,
            tc.tile_pool(name="bp", bufs=6) as bp,
            tc.tile_pool(name="tp", bufs=3) as tp,
            tc.tile_pool(name="op", bufs=2) as op,
        ):
            xr = x.rearrange("h c b w -> h c (b w)")
            X = xp.tile([H, C * BPC * W], f32)
            Xv = X.rearrange("p (c b w) -> p c b w", c=C, b=BPC)
            Xf = X.rearrange("p (c q) -> p c q", c=C)
            half = C // 2
            nc.sync.dma_start(out=Xf[:, 0:half], in_=xr[:, 0:half])
            nc.sync.dma_start(out=Xf[:, half:C], in_=xr[:, half:C])

            S = xp.tile([H, ncols * BPC * W], f32)
            Sv = S.rearrange("p (j b w) -> p j b w", j=ncols, b=BPC)
            Sf = S.rearrange("p (j q) -> p j q", j=ncols)
            # finite filler for shifted-run head/tail rows (junk lanes only)
            for d0 in range(0, ncols, C):
                n = min(C, ncols - d0)
                nc.sync.dma_start(out=Sf[0:2, d0 : d0 + n], in_=xr[0:2, 0:n])
                nc.sync.dma_start(out=Sf[H - 2 : H, d0 : d0 + n], in_=xr[0:2, 0:n])
            for (s, c0, cl, d0) in runlist:
                # S[p, d0+i] = x[c0+i, p+s]; contiguous n-KB run per partition.
                n = cl - c0 + 1
                if s < 0:
                    nc.sync.dma_start(
                        out=Sf[-s:H, d0 : d0 + n], in_=xr[0 : H + s, c0 : c0 + n]
                    )
                else:
                    nc.sync.dma_start(
                        out=Sf[0 : H - s, d0 : d0 + n], in_=xr[s:H, c0 : c0 + n]
                    )

            Gc = xp.tile([H, K], f32)
            nc.sync.dma_start(out=Gc, in_=gcd[:, :])

            outv = out.rearrange("oh k b w -> oh k (b w)")
            fd = BPC * OW

            def emit_gamma_and_store(g, ks, geng, T, O):
                # deferred one group so cross-engine waits are pre-satisfied
                for j, k in enumerate(ks):
                    _, base, _, _, path, scal, gamma = plans[k]
                    cnt = base + OH
                    slot = T[0:cnt, j * fd : (j + 1) * fd]
                    if gamma != 0.0 or geng == "gp":
                        pos = g * GRP + j
                        if geng == "act":
                            nc.scalar.activation(
                                slot, slot, Copy, bias=gamma, scale=1.0
                            )
                        elif geng == "dve":
                            # odd innermost dim forces 1x mode: no shared-port
                            # contention with GpSimd
                            so = slot.rearrange("p (a q) -> p a q", a=4)
                            nc.vector.tensor_scalar(so, so, gamma, None, add)
                        else:
                            gb = Gc[0:cnt, pos : pos + 1].broadcast_to([cnt, fd])
                            osl = O[0:cnt, j * fd : (j + 1) * fd]
                            nc.gpsimd.tensor_tensor(osl, slot, gb, add)
                # batched stores per same-base run: one contiguous-run DMA per
                # run via SWDGE on the GpSimd queue (async transfer).
                src_t = O if geng == "gp" else T
                i = 0
                while i < len(ks):
                    base = plans[ks[i]][1]
                    i2 = i
                    while i2 < len(ks) and plans[ks[i2]][1] == base:
                        i2 += 1
                    src = src_t[base : base + OH, i * fd : i2 * fd].rearrange(
                        "p (kb q) -> p kb q", q=fd
                    )
                    dst = outv[0:OH, g * GRP + i : g * GRP + i2]
                    nc.gpsimd.dma_start(out=dst, in_=src)
                    i = i2

            pending = None
            for g in range(ngrp):
                ks = order[g * GRP : (g + 1) * GRP]
                geng = GSPLIT[g % len(GSPLIT)]
                T = tp.tile([H, GRP * fd], f32, tag="t", name=f"t_{g}")
                O = None
                if geng == "gp":
                    O = op.tile([H, GRP * fd], f32, tag="o", name=f"o_{g}")

                for j, k in enumerate(ks):
                    _, base, a_src, b_src, path, scal, gamma = plans[k]
                    cnt = base + OH

                    def view(src):
                        shifted, idx, woff = src
                        t = Sv if shifted else Xv
                        return t[0:cnt, idx, :, woff : woff + OW]

                    Av, Bv = view(a_src), view(b_src)
                    slot = T[0:cnt, j * fd : (j + 1) * fd]
                    slotv = slot.rearrange("p (b w) -> p b w", b=BPC)
                    b2 = bp.tile([H, fd], f32, tag="b2", name=f"b2_{k}")
                    b2v = b2.rearrange("p (b w) -> p b w", b=BPC)[0:cnt]

                    if path == "fact":
                        kab, kka, alpha = scal
                        nc.scalar.activation(b2v, Bv, Copy, bias=kka, scale=kab)
                        nc.vector.scalar_tensor_tensor(slotv, Av, alpha, b2v, add, mult)
                    else:  # linear/exact: slot = Ca*A + (Cb*B + C1)
                        if path == "linear":
                            kka, kkb, kk1 = scal
                        else:
                            kab, kka, kkb, kk1 = scal
                        nc.scalar.activation(b2v, Bv, Copy, bias=kk1, scale=kkb)
                        nc.vector.scalar_tensor_tensor(slotv, Av, kka, b2v, mult, add)
                        if path == "exact":  # += (Cab*B)*A
                            p2 = bp.tile([H, fd], f32, tag="b2", name=f"p2_{k}")
                            p2v = p2.rearrange("p (b w) -> p b w", b=BPC)[0:cnt]
                            nc.vector.scalar_tensor_tensor(p2v, Bv, kab, Av, mult, mult)
                            nc.vector.tensor_tensor(slot, slot, p2[0:cnt], add)

                if pending is not None:
                    emit_gamma_and_store(*pending)
                pending = (g, ks, geng, T, O)
            if pending is not None:
                emit_gamma_and_store(*pending)
    nc.compile()
    return nc


def _make(x, pairs_a, pairs_b, weights):
    """Build program + per-core input maps + unshard fn (shared with test)."""
    x = np.ascontiguousarray(np.asarray(x), dtype=np.float32)
    pa = np.asarray(pairs_a).astype(np.int64)
    pb = np.asarray(pairs_b).astype(np.int64)
    w = np.asarray(weights).astype(np.float32)

    nc = _build(pa, pb, w)
    _, _, order, gcol = _plan(pa, pb, w)
    in_maps = [
        {
            # [BPC, C, H, W] -> [H, C, BPC, W]
            "x": np.ascontiguousarray(
                x[i * BPC : (i + 1) * BPC].transpose(2, 1, 0, 3)
            ),
            "gcol": gcol,
        }
        for i in range(NCORES)
    ]

    pos = np.empty(K, np.int64)
    pos[np.asarray(order)] = np.arange(K)

    def unshard(results):
        # device layout [OH, K(sorted), BPC, OW] per core -> [B, K, OH, OW]
        full = np.concatenate(
            [r["out"] for r in results], axis=2
        )  # [OH, K, B, OW]
        return np.ascontiguousarray(full[:, pos].transpose(2, 1, 0, 3))

    return nc, in_maps, unshard


def kernel(x, pairs_a, pairs_b, weights):
    from concourse.bass_utils import run_bass_kernel_spmd

    nc, in_maps, unshard = _make(x, pairs_a, pairs_b, weights)
    res = run_bass_kernel_spmd(nc, in_maps, core_ids=list(range(NCORES)))
    return unshard(res.results)


# revision 5
# speedup vs baseline: 2.1579x; 1.9164x over previous
"""Trainium2 Bass kernel for nn_LogicConvSparseMatrix.

Math: the reference's 15-term weighted logic-op sum collapses to

    out[b,k] = C_ab[k]*A*B + C_a[k]*A + C_b[k]*B + C_1[k]

where A = x[b, ca_k, ha_k+oh, wa_k+ow], B = x[b, cb_k, hb_k+oh, wb_k+ow]
are shifted 126x126 windows.  With alpha = C_b/C_ab, gamma = C_1 -
C_a*C_b/C_ab this factors into

    out = (A + alpha) * (C_ab*B + C_a) + gamma

Per kernel k (three element passes; two ops cannot carry 4 coefficients):
  1. ScalarE affine:  B2 = C_ab*B + C_a
  2. VectorE scalar_tensor_tensor:  T = (A + alpha) * B2
  3. "+gamma", load-balanced per group of 8 k's across ScalarE / VectorE /
     GpSimd.

Index pairs are known at build time, so gathers are compile-time SBUF
views of X[p=h, (c,b,w)].  Compute-engine SBUF operands may only start
at partition 0/32/64/96; the relative h-shift between the two windows is
materialized as shifted column copies loaded straight from DRAM,
consolidated into gap-bridged contiguous channel-range runs.

DMA layout (the v1 bottleneck was 1KB load descriptors skewed onto one
SDMA engine): x arrives host-transposed as [H, C, BPC, W] so every load
descriptor is a per-partition CONTIGUOUS run (64KB for the main staging
load, n KB for shifted runs).  The device output layout is
[OH, K(sorted), BPC, OW] so each group store is one ~8KB contiguous run
per partition (~1MB per store DMA).  Stores issue from the GpSimd queue
via SWDGE (async); loads from the Sync queue via HWDGE.
Sharding: data-parallel over batch, 2 batch items per core, 8 cores.
"""

import numpy as np

B, C, H, W = 16, 64, 128, 128
K = 128
RH = RW = 3
OH, OW = H - RH + 1, W - RW + 1
NCORES = 8
BPC = B // NCORES

GRP = 8  # kernels per store group
GSPLIT = ("gp", "gp", "dve", "act")  # gamma-engine per group, round-robin


def _coeffs(weights):
    """Per-kernel coefficients of out = Cab*a*b + Ca*a + Cb*b + C1."""
    w = [weights[:, i].astype(np.float64) for i in range(16)]
    cab = w[1] - w[2] - w[4] - 2 * w[6] - w[7] + w[8] + 2 * w[9] + w[11] + w[13] - w[14]
    ca = w[2] + w[3] + w[6] + w[7] - w[8] - w[9] - w[12] - w[13]
    cb = w[4] + w[5] + w[6] + w[7] - w[8] - w[9] - w[10] - w[11]
    c1 = w[8] + w[9] + w[10] + w[11] + w[12] + w[13] + w[14] + w[15]
    return cab, ca, cb, c1


def _plan(pairs_a, pairs_b, weights):
    """Host-side schedule.  Returns (plans, runs, order, gcol) where
    plans[k] = (k, base, a_src, b_src, path, scal, gamma) with
    a_src/b_src = (from_shifted, column_index, w_off), runs = list of
    (shift, c0, c1, dest_col0) shifted-copy DMAs plus total column count,
    order = base-sorted k order, gcol = broadcast gamma table."""
    cab, ca, cb, c1 = _coeffs(weights)
    keys = {}  # (shift, chan) -> use count; shift != 0
    raw = []
    for k in range(K):
        ha, wa, cca = int(pairs_a[k][0]), int(pairs_a[k][1]), int(pairs_a[k][2])
        hb, wb, ccb = int(pairs_b[k][0]), int(pairs_b[k][1]), int(pairs_b[k][2])
        if ha == hb:
            base = ha
            a_key, b_key = (0, cca), (0, ccb)
        else:
            # shifting either side keeps that copy's invalid rows inside the
            # junk-lane range (min_h + |delta| <= 2); reuse existing columns.
            if ha < hb:  # a is the smaller-h side
                neg = ((ha - hb, cca), True, hb)  # (col key, shifts_a, base)
                pos = ((hb - ha, ccb), False, ha)
            else:
                neg = ((hb - ha, ccb), False, ha)
                pos = ((ha - hb, cca), True, hb)
            key, shift_a, base = pos if (pos[0] in keys and neg[0] not in keys) else neg
            keys[key] = keys.get(key, 0) + 1
            if shift_a:
                a_key, b_key = key, (0, ccb)
            else:
                a_key, b_key = (0, cca), key

        kab, kka, kkb, kk1 = float(cab[k]), float(ca[k]), float(cb[k]), float(c1[k])
        if abs(kab) <= 1e-7:
            path, scal, gamma = "linear", (kka, kkb, kk1), 0.0
        elif abs(kkb) <= 50.0 * abs(kab) and abs(kka * kkb) <= 50.0 * abs(kab):
            path = "fact"
            scal = (kab, kka, kkb / kab)
            gamma = kk1 - kka * kkb / kab
        else:
            path, scal, gamma = "exact", (kab, kka, kkb, kk1), 0.0
        raw.append((k, base, a_key, wa, b_key, wb, path, scal, gamma))

    # consolidate shifted columns into gap-bridged contiguous c-runs
    def build_runs(gaptol):
        runs, cmap, total = [], {}, 0
        for s in sorted({sc[0] for sc in keys}):
            cs = sorted(c for (s2, c) in keys if s2 == s)
            i = 0
            while i < len(cs):
                j = i
                while j + 1 < len(cs) and cs[j + 1] - cs[j] <= gaptol:
                    j += 1
                c0, cl = cs[i], cs[j]
                for c in range(c0, cl + 1):
                    cmap[(s, c)] = total + (c - c0)
                runs.append((s, c0, cl, total))
                total += cl - c0 + 1
                i = j + 1
        return runs, cmap, total

    for gaptol in (8, 4, 1, 0):
        runlist, cmap, ncols = build_runs(gaptol)
        if ncols <= 75:
            break

    plans = []
    for (k, base, a_key, wa, b_key, wb, path, scal, gamma) in raw:
        a_src = (False, a_key[1], wa) if a_key[0] == 0 else (True, cmap[a_key], wa)
        b_src = (False, b_key[1], wb) if b_key[0] == 0 else (True, cmap[b_key], wb)
        plans.append((k, base, a_src, b_src, path, scal, gamma))

    order = sorted(
        range(K), key=lambda k: (plans[k][1], plans[k][2][0] or plans[k][3][0], k)
    )  # by base, no-shift kernels first within each base run
    gcol = np.zeros((H, K), np.float32)
    for pos, k in enumerate(order):
        gcol[:, pos] = plans[k][6]
    return plans, (runlist, ncols), order, gcol


def _build(pairs_a, pairs_b, weights):
    import concourse.bacc as bacc
    import concourse.mybir as mybir
    from concourse.tile import TileContext

    f32 = mybir.dt.float32
    Copy = mybir.ActivationFunctionType.Copy
    add, mult = mybir.AluOpType.add, mybir.AluOpType.mult

    plans, (runlist, ncols), order, gcol_np = _plan(pairs_a, pairs_b, weights)
    ncols = max(1, ncols)
    ngrp = (K + GRP - 1) // GRP

    if ncols > 80:
        raise RuntimeError(f"shifted-column budget exceeded: {ncols}")

    nc = bacc.Bacc()
    # x host-transposed to [H+4, C, BPC, W] with 2 zero pad rows top/bottom:
    # per-partition (h) loads are contiguous DRAM runs (few large DMA
    # descriptors) and every load - main staging AND shifted runs - is a
    # full-128-partition transfer (non-128-partition HWDGE loads were
    # observed to skew their descriptors onto SDMA engine 0).
    x = nc.dram_tensor("x", [H + 4, C, BPC, W], f32, kind="ExternalInput")
    gcd = nc.dram_tensor("gcol", [H, K], f32, kind="ExternalInput")
    # output [OH, K, BPC, OW]: a group store is one contiguous run per
    # partition (oh).  Host inverse-permutes k and transposes.
    out = nc.dram_tensor("out", [OH, K, BPC, OW], f32, kind="ExternalOutput")

    with TileContext(nc) as tc:
        with (
            tc.tile_pool(name="xp", bufs=1) as xp,
            tc.tile_pool(name="bp", bufs=6) as bp,
            tc.tile_pool(name="tp", bufs=3) as tp,
            tc.tile_pool(name="op", bufs=2) as op,
        ):
            xr = x.rearrange("h c b w -> h c (b w)")
            X = xp.tile([H, C * BPC * W], f32)
            Xv = X.rearrange("p (c b w) -> p c b w", c=C, b=BPC)
            Xf = X.rearrange("p (c q) -> p c q", c=C)
            half = C // 2
            nc.sync.dma_start(out=Xf[:, 0:half], in_=xr[2 : 2 + H, 0:half])
            nc.sync.dma_start(out=Xf[:, half:C], in_=xr[2 : 2 + H, half:C])

            S = xp.tile([H, ncols * BPC * W], f32)
            Sv = S.rearrange("p (j b w) -> p j b w", j=ncols, b=BPC)
            Sf = S.rearrange("p (j q) -> p j q", j=ncols)
            for (s, c0, cl, d0) in runlist:
                # S[p, d0+i] = x[c0+i, p+s]; contiguous n-KB run per
                # partition, full 128 partitions via the pad rows (head/tail
                # junk lanes land on pad data; consumers never read them).
                n = cl - c0 + 1
                nc.sync.dma_start(
                    out=Sf[0:H, d0 : d0 + n],
                    in_=xr[2 + s : 2 + s + H, c0 : c0 + n],
                )

            Gc = xp.tile([H, K], f32)
            nc.sync.dma_start(out=Gc, in_=gcd[:, :])

            outv = out.rearrange("oh k b w -> oh k (b w)")
            fd = BPC * OW

            def emit_gamma_and_store(g, ks, geng, T, O):
                # deferred one group so cross-engine waits are pre-satisfied
                for j, k in enumerate(ks):
                    _, base, _, _, path, scal, gamma = plans[k]
                    cnt = base + OH
                    slot = T[0:cnt, j * fd : (j + 1) * fd]
                    if gamma != 0.0 or geng == "gp":
                        pos = g * GRP + j
                        if geng == "act":
                            nc.scalar.activation(
                                slot, slot, Copy, bias=gamma, scale=1.0
                            )
                        elif geng == "dve":
                            # odd innermost dim forces 1x mode: no shared-port
                            # contention with GpSimd
                            so = slot.rearrange("p (a q) -> p a q", a=4)
                            nc.vector.tensor_scalar(so, so, gamma, None, add)
                        else:
                            gb = Gc[0:cnt, pos : pos + 1].broadcast_to([cnt, fd])
                            osl = O[0:cnt, j * fd : (j + 1) * fd]
                            nc.gpsimd.tensor_tensor(osl, slot, gb, add)
                # batched stores per same-base run: one contiguous-run DMA per
                # run via SWDGE on the GpSimd queue (async transfer).
                src_t = O if geng == "gp" else T
                i = 0
                while i < len(ks):
                    base = plans[ks[i]][1]
                    i2 = i
                    while i2 < len(ks) and plans[ks[i2]][1] == base:
                        i2 += 1
                    src = src_t[base : base + OH, i * fd : i2 * fd].rearrange(
                        "p (kb q) -> p kb q", q=fd
                    )
                    dst = outv[0:OH, g * GRP + i : g * GRP + i2]
                    nc.gpsimd.dma_start(out=dst, in_=src)
                    i = i2

            pending = None
            for g in range(ngrp):
                ks = order[g * GRP : (g + 1) * GRP]
                geng = GSPLIT[g % len(GSPLIT)]
                T = tp.tile([H, GRP * fd], f32, tag="t", name=f"t_{g}")
                O = None
                if geng == "gp":
                    O = op.tile([H, GRP * fd], f32, tag="o", name=f"o_{g}")

                for j, k in enumerate(ks):
                    _, base, a_src, b_src, path, scal, gamma = plans[k]
                    cnt = base + OH

                    def view(src):
                        shifted, idx, woff = src
                        t = Sv if shifted else Xv
                        return t[0:cnt, idx, :, woff : woff + OW]

                    Av, Bv = view(a_src), view(b_src)
                    slot = T[0:cnt, j * fd : (j + 1) * fd]
                    slotv = slot.rearrange("p (b w) -> p b w", b=BPC)
                    b2 = bp.tile([H, fd], f32, tag="b2", name=f"b2_{k}")
                    b2v = b2.rearrange("p (b w) -> p b w", b=BPC)[0:cnt]

                    if path == "fact":
                        kab, kka, alpha = scal
                        nc.scalar.activation(b2v, Bv, Copy, bias=kka, scale=kab)
                        nc.vector.scalar_tensor_tensor(slotv, Av, alpha, b2v, add, mult)
                    else:  # linear/exact: slot = Ca*A + (Cb*B + C1)
                        if path == "linear":
                            kka, kkb, kk1 = scal
                        else:
                            kab, kka, kkb, kk1 = scal
                        nc.scalar.activation(b2v, Bv, Copy, bias=kk1, scale=kkb)
                        nc.vector.scalar_tensor_tensor(slotv, Av, kka, b2v, mult, add)
                        if path == "exact":  # += (Cab*B)*A
                            p2 = bp.tile([H, fd], f32, tag="b2", name=f"p2_{k}")
                            p2v = p2.rearrange("p (b w) -> p b w", b=BPC)[0:cnt]
                            nc.vector.scalar_tensor_tensor(p2v, Bv, kab, Av, mult, mult)
                            nc.vector.tensor_tensor(slot, slot, p2[0:cnt], add)

                if pending is not None:
                    emit_gamma_and_store(*pending)
                pending = (g, ks, geng, T, O)
            if pending is not None:
                emit_gamma_and_store(*pending)
    nc.compile()
    return nc


def _make(x, pairs_a, pairs_b, weights):
    """Build program + per-core input maps + unshard fn (shared with test)."""
    x = np.ascontiguousarray(np.asarray(x), dtype=np.float32)
    pa = np.asarray(pairs_a).astype(np.int64)
    pb = np.asarray(pairs_b).astype(np.int64)
    w = np.asarray(weights).astype(np.float32)

    nc = _build(pa, pb, w)
    _, _, order, gcol = _plan(pa, pb, w)

    def xshard(i):
        # [BPC, C, H, W] -> [H+4, C, BPC, W] with 2 zero pad rows each end
        xt = x[i * BPC : (i + 1) * BPC].transpose(2, 1, 0, 3)
        xp = np.zeros((H + 4,) + xt.shape[1:], np.float32)
        xp[2 : 2 + H] = xt
        return xp

    in_maps = [{"x": xshard(i), "gcol": gcol} for i in range(NCORES)]

    pos = np.empty(K, np.int64)
    pos[np.asarray(order)] = np.arange(K)

    def unshard(results):
        # device layout [OH, K(sorted), BPC, OW] per core -> [B, K, OH, OW]
        full = np.concatenate(
            [r["out"] for r in results], axis=2
        )  # [OH, K, B, OW]
        return np.ascontiguousarray(full[:, pos].transpose(2, 1, 0, 3))

    return nc, in_maps, unshard


def kernel(x, pairs_a, pairs_b, weights):
    from concourse.bass_utils import run_bass_kernel_spmd

    nc, in_maps, unshard = _make(x, pairs_a, pairs_b, weights)
    res = run_bass_kernel_spmd(nc, in_maps, core_ids=list(range(NCORES)))
    return unshard(res.results)


# revision 6
# speedup vs baseline: 2.3020x; 1.0668x over previous
"""Trainium2 Bass kernel for nn_LogicConvSparseMatrix.

Math: the reference's 15-term weighted logic-op sum collapses to

    out[b,k] = C_ab[k]*A*B + C_a[k]*A + C_b[k]*B + C_1[k]

where A = x[b, ca_k, ha_k+oh, wa_k+ow], B = x[b, cb_k, hb_k+oh, wb_k+ow]
are shifted 126x126 windows.  With alpha = C_b/C_ab, gamma = C_1 -
C_a*C_b/C_ab this factors into

    out = (A + alpha) * (C_ab*B + C_a) + gamma

Per kernel k (three element passes; two ops cannot carry 4 coefficients):
  1. ScalarE affine:  B2 = C_ab*B + C_a
  2. VectorE scalar_tensor_tensor:  T = (A + alpha) * B2
  3. "+gamma", load-balanced per group of 8 k's across ScalarE / VectorE /
     GpSimd.

Index pairs are known at build time, so gathers are compile-time SBUF
views of X[p=h, (c,b,w)].  Compute-engine SBUF operands may only start
at partition 0/32/64/96; the relative h-shift between the two windows is
materialized as shifted column copies loaded straight from DRAM,
consolidated into gap-bridged contiguous channel-range runs.

DMA layout (the v1 bottleneck was 1KB load descriptors skewed onto one
SDMA engine): x arrives host-transposed as [H, C, BPC, W] so every load
descriptor is a per-partition CONTIGUOUS run (64KB for the main staging
load, n KB for shifted runs).  The device output layout is
[OH, K(sorted), BPC, OW] so each group store is one ~8KB contiguous run
per partition (~1MB per store DMA).  Stores issue from the GpSimd queue
via SWDGE (async); loads from the Sync queue via HWDGE.
Sharding: data-parallel over batch, 2 batch items per core, 8 cores.
"""

import numpy as np

B, C, H, W = 16, 64, 128, 128
K = 128
RH = RW = 3
OH, OW = H - RH + 1, W - RW + 1
NCORES = 8
BPC = B // NCORES

GRP = 8  # kernels per store group
GSPLIT = ("gp", "gp", "dve", "act")  # gamma-engine per group, round-robin


def _coeffs(weights):
    """Per-kernel coefficients of out = Cab*a*b + Ca*a + Cb*b + C1."""
    w = [weights[:, i].astype(np.float64) for i in range(16)]
    cab = w[1] - w[2] - w[4] - 2 * w[6] - w[7] + w[8] + 2 * w[9] + w[11] + w[13] - w[14]
    ca = w[2] + w[3] + w[6] + w[7] - w[8] - w[9] - w[12] - w[13]
    cb = w[4] + w[5] + w[6] + w[7] - w[8] - w[9] - w[10] - w[11]
    c1 = w[8] + w[9] + w[10] + w[11] + w[12] + w[13] + w[14] + w[15]
    return cab, ca, cb, c1


def _plan(pairs_a, pairs_b, weights):
    """Host-side schedule.  Returns (plans, runs, order, gcol) where
    plans[k] = (k, base, a_src, b_src, path, scal, gamma) with
    a_src/b_src = (from_shifted, column_index, w_off), runs = list of
    (shift, c0, c1, dest_col0) shifted-copy DMAs plus total column count,
    order = base-sorted k order, gcol = broadcast gamma table."""
    cab, ca, cb, c1 = _coeffs(weights)
    keys = {}  # (shift, chan) -> use count; shift != 0
    raw = []
    for k in range(K):
        ha, wa, cca = int(pairs_a[k][0]), int(pairs_a[k][1]), int(pairs_a[k][2])
        hb, wb, ccb = int(pairs_b[k][0]), int(pairs_b[k][1]), int(pairs_b[k][2])
        if ha == hb:
            base = ha
            a_key, b_key = (0, cca), (0, ccb)
        else:
            # shifting either side keeps that copy's invalid rows inside the
            # junk-lane range (min_h + |delta| <= 2); reuse existing columns.
            if ha < hb:  # a is the smaller-h side
                neg = ((ha - hb, cca), True, hb)  # (col key, shifts_a, base)
                pos = ((hb - ha, ccb), False, ha)
            else:
                neg = ((hb - ha, ccb), False, ha)
                pos = ((ha - hb, cca), True, hb)
            key, shift_a, base = pos if (pos[0] in keys and neg[0] not in keys) else neg
            keys[key] = keys.get(key, 0) + 1
            if shift_a:
                a_key, b_key = key, (0, ccb)
            else:
                a_key, b_key = (0, cca), key

        kab, kka, kkb, kk1 = float(cab[k]), float(ca[k]), float(cb[k]), float(c1[k])
        if abs(kab) <= 1e-7:
            path, scal, gamma = "linear", (kka, kkb, kk1), 0.0
        elif abs(kkb) <= 50.0 * abs(kab) and abs(kka * kkb) <= 50.0 * abs(kab):
            path = "fact"
            scal = (kab, kka, kkb / kab)
            gamma = kk1 - kka * kkb / kab
        else:
            path, scal, gamma = "exact", (kab, kka, kkb, kk1), 0.0
        raw.append((k, base, a_key, wa, b_key, wb, path, scal, gamma))

    # consolidate shifted columns into gap-bridged contiguous c-runs
    def build_runs(gaptol):
        runs, cmap, total = [], {}, 0
        for s in sorted({sc[0] for sc in keys}):
            cs = sorted(c for (s2, c) in keys if s2 == s)
            i = 0
            while i < len(cs):
                j = i
                while j + 1 < len(cs) and cs[j + 1] - cs[j] <= gaptol:
                    j += 1
                c0, cl = cs[i], cs[j]
                for c in range(c0, cl + 1):
                    cmap[(s, c)] = total + (c - c0)
                runs.append((s, c0, cl, total))
                total += cl - c0 + 1
                i = j + 1
        return runs, cmap, total

    for gaptol in (8, 4, 1, 0):
        runlist, cmap, ncols = build_runs(gaptol)
        if ncols <= 75:
            break

    plans = []
    for (k, base, a_key, wa, b_key, wb, path, scal, gamma) in raw:
        a_src = (False, a_key[1], wa) if a_key[0] == 0 else (True, cmap[a_key], wa)
        b_src = (False, b_key[1], wb) if b_key[0] == 0 else (True, cmap[b_key], wb)
        plans.append((k, base, a_src, b_src, path, scal, gamma))

    order = sorted(
        range(K), key=lambda k: (plans[k][1], plans[k][2][0] or plans[k][3][0], k)
    )  # by base, no-shift kernels first within each base run
    gcol = np.zeros((H, K), np.float32)
    for pos, k in enumerate(order):
        gcol[:, pos] = plans[k][6]
    return plans, (runlist, ncols), order, gcol


def _build(pairs_a, pairs_b, weights):
    import concourse.bacc as bacc
    import concourse.mybir as mybir
    from concourse.tile import TileContext

    f32 = mybir.dt.float32
    bf16 = mybir.dt.bfloat16
    Copy = mybir.ActivationFunctionType.Copy
    add, mult = mybir.AluOpType.add, mybir.AluOpType.mult

    plans, (runlist, ncols), order, gcol_np = _plan(pairs_a, pairs_b, weights)
    ncols = max(1, ncols)
    ngrp = (K + GRP - 1) // GRP

    if ncols > 80:
        raise RuntimeError(f"shifted-column budget exceeded: {ncols}")

    nc = bacc.Bacc()
    # x host-transposed to [H+4, C, BPC, W] with 2 zero pad rows top/bottom:
    # per-partition (h) loads are contiguous DRAM runs (few large DMA
    # descriptors) and every load - main staging AND shifted runs - is a
    # full-128-partition transfer (non-128-partition HWDGE loads were
    # observed to skew their descriptors onto SDMA engine 0).
    x = nc.dram_tensor("x", [H + 4, C, BPC, W], bf16, kind="ExternalInput")
    gcd = nc.dram_tensor("gcol", [H, K], f32, kind="ExternalInput")
    # output [OH, K, BPC, OW]: a group store is one contiguous run per
    # partition (oh).  Host inverse-permutes k and transposes.
    out = nc.dram_tensor("out", [OH, K, BPC, OW], bf16, kind="ExternalOutput")

    with TileContext(nc) as tc:
        with (
            tc.tile_pool(name="xp", bufs=1) as xp,
            tc.tile_pool(name="bp", bufs=6) as bp,
            tc.tile_pool(name="tp", bufs=3) as tp,
            tc.tile_pool(name="op", bufs=2) as op,
        ):
            xr = x.rearrange("h c b w -> h c (b w)")
            X = xp.tile([H, C * BPC * W], bf16)
            Xv = X.rearrange("p (c b w) -> p c b w", c=C, b=BPC)
            Xf = X.rearrange("p (c q) -> p c q", c=C)
            half = C // 2
            nc.sync.dma_start(out=Xf[:, 0:half], in_=xr[2 : 2 + H, 0:half])
            nc.sync.dma_start(out=Xf[:, half:C], in_=xr[2 : 2 + H, half:C])

            S = xp.tile([H, ncols * BPC * W], bf16)
            Sv = S.rearrange("p (j b w) -> p j b w", j=ncols, b=BPC)
            Sf = S.rearrange("p (j q) -> p j q", j=ncols)
            for (s, c0, cl, d0) in runlist:
                # S[p, d0+i] = x[c0+i, p+s]; contiguous n-KB run per
                # partition, full 128 partitions via the pad rows (head/tail
                # junk lanes land on pad data; consumers never read them).
                n = cl - c0 + 1
                nc.sync.dma_start(
                    out=Sf[0:H, d0 : d0 + n],
                    in_=xr[2 + s : 2 + s + H, c0 : c0 + n],
                )

            Gc = xp.tile([H, K], f32)
            nc.sync.dma_start(out=Gc, in_=gcd[:, :])

            outv = out.rearrange("oh k b w -> oh k (b w)")
            fd = BPC * OW

            def emit_gamma_and_store(g, ks, geng, T, O):
                # deferred one group so cross-engine waits are pre-satisfied
                for j, k in enumerate(ks):
                    _, base, _, _, path, scal, gamma = plans[k]
                    cnt = base + OH
                    slot = T[0:cnt, j * fd : (j + 1) * fd]
                    if gamma != 0.0 or geng == "gp":
                        pos = g * GRP + j
                        if geng == "act":
                            nc.scalar.activation(
                                slot, slot, Copy, bias=gamma, scale=1.0
                            )
                        elif geng == "dve":
                            # odd innermost dim forces 1x mode: no shared-port
                            # contention with GpSimd
                            so = slot.rearrange("p (a q) -> p a q", a=4)
                            nc.vector.tensor_scalar(so, so, gamma, None, add)
                        else:
                            gb = Gc[0:cnt, pos : pos + 1].broadcast_to([cnt, fd])
                            osl = O[0:cnt, j * fd : (j + 1) * fd]
                            nc.gpsimd.tensor_tensor(osl, slot, gb, add)
                # batched stores per same-base run: one contiguous-run DMA per
                # run via SWDGE on the GpSimd queue (async transfer).
                src_t = O if geng == "gp" else T
                i = 0
                while i < len(ks):
                    base = plans[ks[i]][1]
                    i2 = i
                    while i2 < len(ks) and plans[ks[i2]][1] == base:
                        i2 += 1
                    src = src_t[base : base + OH, i * fd : i2 * fd].rearrange(
                        "p (kb q) -> p kb q", q=fd
                    )
                    dst = outv[0:OH, g * GRP + i : g * GRP + i2]
                    nc.gpsimd.dma_start(out=dst, in_=src)
                    i = i2

            pending = None
            for g in range(ngrp):
                ks = order[g * GRP : (g + 1) * GRP]
                geng = GSPLIT[g % len(GSPLIT)]
                T = tp.tile([H, GRP * fd], bf16, tag="t", name=f"t_{g}")
                O = None
                if geng == "gp":
                    O = op.tile([H, GRP * fd], bf16, tag="o", name=f"o_{g}")

                for j, k in enumerate(ks):
                    _, base, a_src, b_src, path, scal, gamma = plans[k]
                    cnt = base + OH

                    def view(src):
                        shifted, idx, woff = src
                        t = Sv if shifted else Xv
                        return t[0:cnt, idx, :, woff : woff + OW]

                    Av, Bv = view(a_src), view(b_src)
                    slot = T[0:cnt, j * fd : (j + 1) * fd]
                    slotv = slot.rearrange("p (b w) -> p b w", b=BPC)
                    b2 = bp.tile([H, fd], bf16, tag="b2", name=f"b2_{k}")
                    b2v = b2.rearrange("p (b w) -> p b w", b=BPC)[0:cnt]

                    if path == "fact":
                        kab, kka, alpha = scal
                        nc.scalar.activation(b2v, Bv, Copy, bias=kka, scale=kab)
                        nc.vector.scalar_tensor_tensor(slotv, Av, alpha, b2v, add, mult)
                    else:  # linear/exact: slot = Ca*A + (Cb*B + C1)
                        if path == "linear":
                            kka, kkb, kk1 = scal
                        else:
                            kab, kka, kkb, kk1 = scal
                        nc.scalar.activation(b2v, Bv, Copy, bias=kk1, scale=kkb)
                        nc.vector.scalar_tensor_tensor(slotv, Av, kka, b2v, mult, add)
                        if path == "exact":  # += (Cab*B)*A
                            p2 = bp.tile([H, fd], bf16, tag="b2", name=f"p2_{k}")
                            p2v = p2.rearrange("p (b w) -> p b w", b=BPC)[0:cnt]
                            nc.vector.scalar_tensor_tensor(p2v, Bv, kab, Av, mult, mult)
                            nc.vector.tensor_tensor(slot, slot, p2[0:cnt], add)

                if pending is not None:
                    emit_gamma_and_store(*pending)
                pending = (g, ks, geng, T, O)
            if pending is not None:
                emit_gamma_and_store(*pending)
    nc.compile()
    return nc


def _make(x, pairs_a, pairs_b, weights):
    """Build program + per-core input maps + unshard fn (shared with test)."""
    x = np.ascontiguousarray(np.asarray(x), dtype=np.float32)
    pa = np.asarray(pairs_a).astype(np.int64)
    pb = np.asarray(pairs_b).astype(np.int64)
    w = np.asarray(weights).astype(np.float32)

    nc = _build(pa, pb, w)
    _, _, order, gcol = _plan(pa, pb, w)

    def xshard(i):
        # [BPC, C, H, W] -> [H+4, C, BPC, W] with 2 zero pad rows each end
        import ml_dtypes

        xt = x[i * BPC : (i + 1) * BPC].transpose(2, 1, 0, 3)
        xp = np.zeros((H + 4,) + xt.shape[1:], ml_dtypes.bfloat16)
        xp[2 : 2 + H] = xt.astype(ml_dtypes.bfloat16)
        return xp

    in_maps = [{"x": xshard(i), "gcol": gcol} for i in range(NCORES)]

    pos = np.empty(K, np.int64)
    pos[np.asarray(order)] = np.arange(K)

    def unshard(results):
        # device layout [OH, K(sorted), BPC, OW] per core -> [B, K, OH, OW]
        full = np.concatenate(
            [r["out"] for r in results], axis=2
        )  # [OH, K, B, OW]
        return np.ascontiguousarray(
            full[:, pos].transpose(2, 1, 0, 3).astype(np.float32)
        )

    return nc, in_maps, unshard


def kernel(x, pairs_a, pairs_b, weights):
    from concourse.bass_utils import run_bass_kernel_spmd

    nc, in_maps, unshard = _make(x, pairs_a, pairs_b, weights)
    res = run_bass_kernel_spmd(nc, in_maps, core_ids=list(range(NCORES)))
    return unshard(res.results)


# revision 7
# speedup vs baseline: 2.9662x; 1.2885x over previous
"""Trainium2 Bass kernel for nn_LogicConvSparseMatrix.

Math: the reference's 15-term weighted logic-op sum collapses to

    out[b,k] = C_ab[k]*A*B + C_a[k]*A + C_b[k]*B + C_1[k]

where A = x[b, ca_k, ha_k+oh, wa_k+ow], B = x[b, cb_k, hb_k+oh, wb_k+ow]
are shifted 126x126 windows.  Per kernel, with P = one operand and Q =
the other (orientation chosen per kernel), this factors into

    out = (Q + alpha) * (C_ab*P + c_p) + gamma

computed in bf16 as three flat element passes over full-W columns:
  1. affine:   b2 = C_ab*colP[wp:wp+FW] + c_p      (ACT or DVE tensor_scalar)
  2. STT:      T  = (colQ[wq:wq+FW] + alpha) * b2  (DVE)
  3. + gamma   (per-group engine: DVE / ACT / GpSimd)

Flat full-W columns: every operand is a contiguous FW = BPC*W element
slice of an SBUF column, the per-kernel w-window offset absorbed into
the slice start.  Positions w in [OW, W) per batch item are junk lanes
(the <=2-element overread past a column lands in them / in the 2-element
tile pad); the host slices w < OW after the full-W store.  Flat 1-D APs
let the DVE packed perf modes (2x STT / 4x tensor_scalar) engage; the
orientation puts the odd w-offset on pass 1 (alignment-immune ACT)
so the STT input stays 4B-aligned where possible.

h-shifts between the two windows are materialized as shifted column
copies loaded straight from DRAM (x is host-padded by 2 rows top and
bottom so every load is a full-128-partition transfer - non-128-row
HWDGE loads skew their descriptors onto SDMA engine 0 - and host-
transposed to [H+4, C, BPC, W] so every load descriptor is a contiguous
per-partition run).  Device output layout [OH, K(sorted), BPC, W] makes
each group store one contiguous ~4KB run per partition; stores issue
from the GpSimd queue via SWDGE (async transfers).
Sharding: data-parallel over batch, 2 batch items per core, 8 cores.
"""

import numpy as np

B, C, H, W = 16, 64, 128, 128
K = 128
RH = RW = 3
OH, OW = H - RH + 1, W - RW + 1
NCORES = 8
BPC = B // NCORES
FW = BPC * W  # flat column width (elements per partition per column)

GRP = 8  # kernels per store group
GSPLIT = ("dve", "act", "gp", "dve")  # gamma-engine per group, round-robin
B2_DVE_WHEN_EVEN = True  # pass-1 affine on DVE (4x TS) when its offset is even


def _coeffs(weights):
    """Per-kernel coefficients of out = Cab*a*b + Ca*a + Cb*b + C1."""
    w = [weights[:, i].astype(np.float64) for i in range(16)]
    cab = w[1] - w[2] - w[4] - 2 * w[6] - w[7] + w[8] + 2 * w[9] + w[11] + w[13] - w[14]
    ca = w[2] + w[3] + w[6] + w[7] - w[8] - w[9] - w[12] - w[13]
    cb = w[4] + w[5] + w[6] + w[7] - w[8] - w[9] - w[10] - w[11]
    c1 = w[8] + w[9] + w[10] + w[11] + w[12] + w[13] + w[14] + w[15]
    return cab, ca, cb, c1


def _plan(pairs_a, pairs_b, weights):
    """Host-side schedule.  plans[k] = (k, base, q_src, p_src, path, scal,
    gamma, b2_dve) with q_src/p_src = (from_shifted, column_index, w_off)
    for the STT side (Q) and affine side (P).  runs = shifted-copy DMA list
    (shift, c0, c1, dest_col0) + total column count; order = base-sorted k
    order; gcol = broadcast gamma table (f32)."""
    cab, ca, cb, c1 = _coeffs(weights)
    keys = {}  # (shift, chan) -> use count; shift != 0
    raw = []
    for k in range(K):
        ha, wa, cca = int(pairs_a[k][0]), int(pairs_a[k][1]), int(pairs_a[k][2])
        hb, wb, ccb = int(pairs_b[k][0]), int(pairs_b[k][1]), int(pairs_b[k][2])
        if ha == hb:
            base = ha
            a_key, b_key = (0, cca), (0, ccb)
        else:
            # shifting either side keeps that copy's invalid rows inside the
            # junk-lane range; reuse existing columns where possible.
            if ha < hb:
                neg = ((ha - hb, cca), True, hb)  # (col key, shifts_a, base)
                pos = ((hb - ha, ccb), False, ha)
            else:
                neg = ((hb - ha, ccb), False, ha)
                pos = ((ha - hb, cca), True, hb)
            key, shift_a, base = pos if (pos[0] in keys and neg[0] not in keys) else neg
            keys[key] = keys.get(key, 0) + 1
            if shift_a:
                a_key, b_key = key, (0, ccb)
            else:
                a_key, b_key = (0, cca), key

        kab, kka, kkb, kk1 = float(cab[k]), float(ca[k]), float(cb[k]), float(c1[k])
        # orientation: P = affine side (absorbs odd offset), Q = STT side.
        # out = (Q + alpha)*(Cab*P + c_p) + gamma; alpha = c_q/Cab.
        # P=B: c_p=Ca, alpha=Cb/Cab.  P=A: c_p=Cb, alpha=Ca/Cab.
        cand = []
        if abs(kab) > 1e-7 and abs(kka * kkb) <= 50.0 * abs(kab):
            if abs(kkb) <= 50.0 * abs(kab):
                cand.append(("B", abs(kkb / kab)))  # P=B, Q=A
            if abs(kka) <= 50.0 * abs(kab):
                cand.append(("A", abs(kka / kab)))  # P=A, Q=B
        if cand:
            # prefer even STT-side offset (DVE 2x); tie-break smaller |alpha|
            def rank(c):
                qoff = wa if c[0] == "B" else wb
                return (qoff % 2, c[1])

            cand.sort(key=rank)
            pside = cand[0][0]
            path = "fact"
            if pside == "B":
                scal = (kab, kka, kkb / kab)
                q_key, qw, p_key, pw = a_key, wa, b_key, wb
            else:
                scal = (kab, kkb, kka / kab)
                q_key, qw, p_key, pw = b_key, wb, a_key, wa
            gamma = kk1 - kka * kkb / kab
        elif abs(kab) <= 1e-7:
            path, scal, gamma = "linear", (kka, kkb, kk1), 0.0
            q_key, qw, p_key, pw = a_key, wa, b_key, wb
        else:
            path, scal, gamma = "exact", (kab, kka, kkb, kk1), 0.0
            q_key, qw, p_key, pw = a_key, wa, b_key, wb
        raw.append((k, base, q_key, qw, p_key, pw, path, scal, gamma))

    # consolidate shifted columns into gap-bridged contiguous c-runs
    def build_runs(gaptol):
        runs, cmap, total = [], {}, 0
        for s in sorted({sc[0] for sc in keys}):
            cs = sorted(c for (s2, c) in keys if s2 == s)
            i = 0
            while i < len(cs):
                j = i
                while j + 1 < len(cs) and cs[j + 1] - cs[j] <= gaptol:
                    j += 1
                c0, cl = cs[i], cs[j]
                for c in range(c0, cl + 1):
                    cmap[(s, c)] = total + (c - c0)
                runs.append((s, c0, cl, total))
                total += cl - c0 + 1
                i = j + 1
        return runs, cmap, total

    for gaptol in (8, 4, 1, 0):
        runlist, cmap, ncols = build_runs(gaptol)
        if ncols <= 75:
            break

    plans = []
    for (k, base, q_key, qw, p_key, pw, path, scal, gamma) in raw:
        q_src = (False, q_key[1], qw) if q_key[0] == 0 else (True, cmap[q_key], qw)
        p_src = (False, p_key[1], pw) if p_key[0] == 0 else (True, cmap[p_key], pw)
        b2_dve = B2_DVE_WHEN_EVEN and path == "fact" and pw % 2 == 0
        plans.append((k, base, q_src, p_src, path, scal, gamma, b2_dve))

    order = sorted(
        range(K), key=lambda k: (plans[k][1], plans[k][2][0] or plans[k][3][0], k)
    )  # by base, no-shift kernels first within each base run
    gcol = np.zeros((H, K), np.float32)
    for pos, k in enumerate(order):
        gcol[:, pos] = plans[k][6]
    return plans, (runlist, ncols), order, gcol


def _build(pairs_a, pairs_b, weights):
    import concourse.bacc as bacc
    import concourse.mybir as mybir
    from concourse.tile import TileContext

    f32 = mybir.dt.float32
    bf16 = mybir.dt.bfloat16
    Copy = mybir.ActivationFunctionType.Copy
    add, mult = mybir.AluOpType.add, mybir.AluOpType.mult

    plans, (runlist, ncols), order, gcol_np = _plan(pairs_a, pairs_b, weights)
    ncols = max(1, ncols)
    ngrp = (K + GRP - 1) // GRP

    if ncols > 80:
        raise RuntimeError(f"shifted-column budget exceeded: {ncols}")

    nc = bacc.Bacc()
    x = nc.dram_tensor("x", [H + 4, C, BPC, W], bf16, kind="ExternalInput")
    gcd = nc.dram_tensor("gcol", [H, K], f32, kind="ExternalInput")
    out = nc.dram_tensor("out", [OH, K, BPC, W], bf16, kind="ExternalOutput")

    with TileContext(nc) as tc:
        with (
            tc.tile_pool(name="xp", bufs=1) as xp,
            tc.tile_pool(name="bp", bufs=10) as bp,
            tc.tile_pool(name="tp", bufs=4) as tp,
            tc.tile_pool(name="op", bufs=3) as op,
        ):
            xr = x.rearrange("h c b w -> h (c b w)")
            # +2 element pad: flat w-offset views overread <=2 elements past
            # the last column; the pad keeps that read in-bounds.
            X = xp.tile([H, C * FW + 2], bf16)
            nc.gpsimd.memset(X[:, C * FW : C * FW + 2], 0.0)
            quarter = C * FW // 4
            for q in range(4):
                nc.sync.dma_start(
                    out=X[:, q * quarter : (q + 1) * quarter],
                    in_=xr[2 : 2 + H, q * quarter : (q + 1) * quarter],
                )

            S = xp.tile([H, ncols * FW + 2], bf16)
            nc.gpsimd.memset(S[:, ncols * FW : ncols * FW + 2], 0.0)
            for (s, c0, cl, d0) in runlist:
                # S[p, d0+i] = x[c0+i, p+s]; contiguous n-KB run per
                # partition, full 128 partitions via the pad rows.
                n = cl - c0 + 1
                nc.sync.dma_start(
                    out=S[0:H, d0 * FW : (d0 + n) * FW],
                    in_=xr[2 + s : 2 + s + H, c0 * FW : (c0 + n) * FW],
                )

            Gc = xp.tile([H, K], f32)
            nc.sync.dma_start(out=Gc, in_=gcd[:, :])

            outv = out.rearrange("oh k b w -> oh (k b w)")

            def col(src, cnt):
                shifted, idx, woff = src
                t = S if shifted else X
                return t[0:cnt, idx * FW + woff : idx * FW + woff + FW]

            def emit_gamma_and_store(g, ks, geng, T, O):
                # deferred one group so cross-engine waits are pre-satisfied
                for j, k in enumerate(ks):
                    _, base, _, _, path, scal, gamma, _ = plans[k]
                    cnt = base + OH
                    slot = T[0:cnt, j * FW : (j + 1) * FW]
                    if gamma != 0.0 or geng == "gp":
                        pos = g * GRP + j
                        if geng == "act":
                            nc.scalar.activation(
                                slot, slot, Copy, bias=gamma, scale=1.0
                            )
                        elif geng == "dve":
                            nc.vector.tensor_scalar(slot, slot, gamma, None, add)
                        else:
                            gb = Gc[0:cnt, pos : pos + 1].broadcast_to([cnt, FW])
                            osl = O[0:cnt, j * FW : (j + 1) * FW]
                            nc.gpsimd.tensor_tensor(osl, slot, gb, add)
                # batched stores per same-base run: one contiguous-run DMA per
                # run via SWDGE on the GpSimd queue (async transfer).
                src_t = O if geng == "gp" else T
                i = 0
                while i < len(ks):
                    base = plans[ks[i]][1]
                    i2 = i
                    while i2 < len(ks) and plans[ks[i2]][1] == base:
                        i2 += 1
                    src = src_t[base : base + OH, i * FW : i2 * FW]
                    dst = outv[
                        0:OH, (g * GRP + i) * FW : (g * GRP + i2) * FW
                    ]
                    nc.gpsimd.dma_start(out=dst, in_=src)
                    i = i2

            pending = None
            for g in range(ngrp):
                ks = order[g * GRP : (g + 1) * GRP]
                geng = GSPLIT[g % len(GSPLIT)]
                T = tp.tile([H, GRP * FW], bf16, tag="t", name=f"t_{g}")
                O = None
                if geng == "gp":
                    O = op.tile([H, GRP * FW], bf16, tag="o", name=f"o_{g}")

                for j, k in enumerate(ks):
                    _, base, q_src, p_src, path, scal, gamma, b2_dve = plans[k]
                    cnt = base + OH
                    Qv, Pv = col(q_src, cnt), col(p_src, cnt)
                    slot = T[0:cnt, j * FW : (j + 1) * FW]
                    b2 = bp.tile([H, FW], bf16, tag="b2", name=f"b2_{k}")
                    b2v = b2[0:cnt]

                    if path == "fact":
                        kab, c_p, alpha = scal
                        if b2_dve:
                            nc.vector.tensor_scalar(b2v, Pv, kab, c_p, mult, add)
                        else:
                            nc.scalar.activation(b2v, Pv, Copy, bias=c_p, scale=kab)
                        nc.vector.scalar_tensor_tensor(slot, Qv, alpha, b2v, add, mult)
                    else:  # linear/exact: slot = Ca*Q + (Cb*P + C1)
                        if path == "linear":
                            kka, kkb, kk1 = scal
                        else:
                            kab, kka, kkb, kk1 = scal
                        nc.scalar.activation(b2v, Pv, Copy, bias=kk1, scale=kkb)
                        nc.vector.scalar_tensor_tensor(slot, Qv, kka, b2v, mult, add)
                        if path == "exact":  # += (Cab*P)*Q
                            p2 = bp.tile([H, FW], bf16, tag="b2", name=f"p2_{k}")
                            p2v = p2[0:cnt]
                            nc.vector.scalar_tensor_tensor(p2v, Pv, kab, Qv, mult, mult)
                            nc.vector.tensor_tensor(slot, slot, p2v, add)

                if pending is not None:
                    emit_gamma_and_store(*pending)
                pending = (g, ks, geng, T, O)
            if pending is not None:
                emit_gamma_and_store(*pending)
    nc.compile()
    return nc


def _make(x, pairs_a, pairs_b, weights):
    """Build program + per-core input maps + unshard fn (shared with test)."""
    import ml_dtypes

    x = np.ascontiguousarray(np.asarray(x), dtype=np.float32)
    pa = np.asarray(pairs_a).astype(np.int64)
    pb = np.asarray(pairs_b).astype(np.int64)
    w = np.asarray(weights).astype(np.float32)

    nc = _build(pa, pb, w)
    _, _, order, gcol = _plan(pa, pb, w)

    def xshard(i):
        # [BPC, C, H, W] -> [H+4, C, BPC, W] with 2 zero pad rows each end
        xt = x[i * BPC : (i + 1) * BPC].transpose(2, 1, 0, 3)
        xp = np.zeros((H + 4,) + xt.shape[1:], ml_dtypes.bfloat16)
        xp[2 : 2 + H] = xt.astype(ml_dtypes.bfloat16)
        return xp

    in_maps = [{"x": xshard(i), "gcol": gcol} for i in range(NCORES)]

    pos = np.empty(K, np.int64)
    pos[np.asarray(order)] = np.arange(K)

    def unshard(results):
        # device layout [OH, K(sorted), BPC, W] per core -> [B, K, OH, OW]
        full = np.concatenate(
            [r["out"] for r in results], axis=2
        )  # [OH, K, B, W]
        return np.ascontiguousarray(
            full[:, pos, :, :OW].transpose(2, 1, 0, 3).astype(np.float32)
        )

    return nc, in_maps, unshard


def kernel(x, pairs_a, pairs_b, weights):
    from concourse.bass_utils import run_bass_kernel_spmd

    nc, in_maps, unshard = _make(x, pairs_a, pairs_b, weights)
    res = run_bass_kernel_spmd(nc, in_maps, core_ids=list(range(NCORES)))
    return unshard(res.results)


# revision 10
# speedup vs baseline: 3.1479x; 1.0612x over previous
"""Trainium2 Bass kernel for nn_LogicConvSparseMatrix.

Math: the reference's 15-term weighted logic-op sum collapses to

    out[b,k] = C_ab[k]*A*B + C_a[k]*A + C_b[k]*B + C_1[k]

where A = x[b, ca_k, ha_k+oh, wa_k+ow], B = x[b, cb_k, hb_k+oh, wb_k+ow]
are shifted 126x126 windows.  Per kernel, with P = one operand and Q =
the other (orientation chosen per kernel), this factors into

    out = (Q + alpha) * (C_ab*P + c_p) + gamma

computed in bf16 as three flat element passes over full-W columns:
  1. affine:   b2 = C_ab*colP[wp:wp+FW] + c_p   (DVE 4x tensor_scalar when
               wp is even, else alignment-immune ACT)
  2. STT:      T  = (colQ[wq:wq+FW] + alpha) * b2  (DVE, 2x packed mode)
  3. + gamma   in place on T (DVE 4x tensor_scalar / ACT copy-bias split;
               never GpSimd - its SBUF ops grab the DVE shared port pair
               and stall the DVE packed modes)

Flat full-W columns: every operand is a contiguous FW = BPC*W element
slice of an SBUF column, the per-kernel w-window offset absorbed into
the slice start.  Positions w in [OW, W) per batch item are junk lanes
(the <=2-element overread past a column lands in them / in the 2-element
tile pad); the host slices w < OW after the full-W store.  DVE packed
perf modes need 4-byte-aligned bf16 operand starts, so odd w-offsets are
handled by (a) routing the affine pass to ACT for odd wp and (b) loading
extra +1-element-preshifted copies of the Q columns whose kernels have
both offsets odd.

h-shifts between the two windows are materialized as shifted column
copies loaded straight from DRAM (x is host-padded by 2 rows top and
bottom so every load is a full-128-partition transfer - non-128-row
HWDGE loads skew their descriptors onto SDMA engine 0 - and host-
transposed/flattened to [H+4, C*BPC*W+2] so every load descriptor is a
contiguous per-partition run).  Device output layout [OH, K(sorted),
BPC, W] makes each group store one contiguous ~4KB run per partition;
stores issue from the GpSimd queue via SWDGE (async transfers).
Sharding: data-parallel over batch, 2 batch items per core, 8 cores.
"""

import numpy as np

B, C, H, W = 16, 64, 128, 128
K = 128
RH = RW = 3
OH, OW = H - RH + 1, W - RW + 1
NCORES = 8
BPC = B // NCORES
FW = BPC * W  # flat column width (elements per partition per column)

GRP = 8  # kernels per store group
B2_DVE_WHEN_EVEN = True  # pass-1 affine on DVE (4x TS) when its offset is even
GAMMA_DVE_RATIO = 1  # of every 2 gamma ops, this many on DVE (rest ACT)


def _coeffs(weights):
    """Per-kernel coefficients of out = Cab*a*b + Ca*a + Cb*b + C1."""
    w = [weights[:, i].astype(np.float64) for i in range(16)]
    cab = w[1] - w[2] - w[4] - 2 * w[6] - w[7] + w[8] + 2 * w[9] + w[11] + w[13] - w[14]
    ca = w[2] + w[3] + w[6] + w[7] - w[8] - w[9] - w[12] - w[13]
    cb = w[4] + w[5] + w[6] + w[7] - w[8] - w[9] - w[10] - w[11]
    c1 = w[8] + w[9] + w[10] + w[11] + w[12] + w[13] + w[14] + w[15]
    return cab, ca, cb, c1


def _plan(pairs_a, pairs_b, weights):
    """Host-side schedule.  plans[k] = (k, base, q_src, p_src, path, scal,
    gamma, b2_dve) with q_src/p_src = (from_shifted, column_index, w_off).
    runs = shifted-copy DMA list (hshift, wshift, c0, c1, dest_col0) +
    total column count; order = base-sorted k order; gcol = gamma table."""
    cab, ca, cb, c1 = _coeffs(weights)
    keys = {}  # (hshift, chan) -> count; hshift != 0 (wshift=0 columns)
    raw = []
    for k in range(K):
        ha, wa, cca = int(pairs_a[k][0]), int(pairs_a[k][1]), int(pairs_a[k][2])
        hb, wb, ccb = int(pairs_b[k][0]), int(pairs_b[k][1]), int(pairs_b[k][2])
        if ha == hb:
            base = ha
            a_key, b_key = (0, cca), (0, ccb)
        else:
            # shifting either side keeps that copy's invalid rows inside the
            # junk-lane range; reuse existing columns where possible.
            if ha < hb:
                neg = ((ha - hb, cca), True, hb)  # (col key, shifts_a, base)
                pos = ((hb - ha, ccb), False, ha)
            else:
                neg = ((hb - ha, ccb), False, ha)
                pos = ((ha - hb, cca), True, hb)
            key, shift_a, base = pos if (pos[0] in keys and neg[0] not in keys) else neg
            keys[key] = keys.get(key, 0) + 1
            if shift_a:
                a_key, b_key = key, (0, ccb)
            else:
                a_key, b_key = (0, cca), key

        kab, kka, kkb, kk1 = float(cab[k]), float(ca[k]), float(cb[k]), float(c1[k])
        # orientation: P = affine side (ACT absorbs odd offset), Q = STT
        # side.  out = (Q + alpha)*(Cab*P + c_p) + gamma; alpha = c_q/Cab.
        cand = []
        if abs(kab) > 1e-7 and abs(kka * kkb) <= 50.0 * abs(kab):
            if abs(kkb) <= 50.0 * abs(kab):
                cand.append(("B", abs(kkb / kab)))  # P=B, Q=A
            if abs(kka) <= 50.0 * abs(kab):
                cand.append(("A", abs(kka / kab)))  # P=A, Q=B
        if cand:
            # prefer even STT-side offset (DVE 2x); tie-break smaller |alpha|
            def rank(c):
                qoff = wa if c[0] == "B" else wb
                return (qoff % 2, c[1])

            cand.sort(key=rank)
            pside = cand[0][0]
            path = "fact"
            if pside == "B":
                scal = (kab, kka, kkb / kab)
                q_key, qw, p_key, pw = a_key, wa, b_key, wb
            else:
                scal = (kab, kkb, kka / kab)
                q_key, qw, p_key, pw = b_key, wb, a_key, wa
            gamma = kk1 - kka * kkb / kab
        elif abs(kab) <= 1e-7:
            path, scal, gamma = "linear", (kka, kkb, kk1), 0.0
            q_key, qw, p_key, pw = a_key, wa, b_key, wb
        else:
            path, scal, gamma = "exact", (kab, kka, kkb, kk1), 0.0
            q_key, qw, p_key, pw = a_key, wa, b_key, wb
        raw.append((k, base, q_key, qw, p_key, pw, path, scal, gamma))

    # Q columns read at odd offsets run the DVE STT at half rate; give them
    # a +1-element-preshifted copy so the read is 4B-aligned.  wkeys values
    # are (hshift, chan) with wshift=1.
    wkeys = {}
    for i, (k, base, q_key, qw, p_key, pw, path, scal, gamma) in enumerate(raw):
        if path in ("fact", "linear") and qw % 2 == 1:
            wkeys[q_key] = wkeys.get(q_key, 0) + 1

    # consolidate shifted columns into gap-bridged contiguous c-runs
    def build_runs(gaptol):
        runs, cmap, wmap, total = [], {}, {}, 0
        for s in sorted({sc[0] for sc in keys}):
            cs = sorted(c for (s2, c) in keys if s2 == s)
            i = 0
            while i < len(cs):
                j = i
                while j + 1 < len(cs) and cs[j + 1] - cs[j] <= gaptol:
                    j += 1
                c0, cl = cs[i], cs[j]
                for c in range(c0, cl + 1):
                    cmap[(s, c)] = total + (c - c0)
                runs.append((s, 0, c0, cl, total))
                total += cl - c0 + 1
                i = j + 1
        for s in sorted({sc[0] for sc in wkeys}):
            cs = sorted(c for (s2, c) in wkeys if s2 == s)
            i = 0
            while i < len(cs):
                j = i
                while j + 1 < len(cs) and cs[j + 1] - cs[j] <= gaptol:
                    j += 1
                c0, cl = cs[i], cs[j]
                for c in range(c0, cl + 1):
                    wmap[(s, c)] = total + (c - c0)
                runs.append((s, 1, c0, cl, total))
                total += cl - c0 + 1
                i = j + 1
        return runs, cmap, wmap, total

    for gaptol in (8, 4, 1, 0):
        runlist, cmap, wmap, ncols = build_runs(gaptol)
        if ncols <= 100:
            break

    def resolve(key, woff, prefer_w):
        # (from_shifted, column_index, w_off) with wshift-1 copy if asked
        if prefer_w and woff % 2 == 1 and key in wmap:
            return (True, wmap[key], woff - 1)
        if key[0] == 0 and not (prefer_w and woff % 2 == 1):
            return (False, key[1], woff)
        if key[0] == 0:
            # raw column but odd offset wanted and no wshift copy: key (0,c)
            # may still be in wmap via wkeys
            if prefer_w and key in wmap:
                return (True, wmap[key], woff - 1)
            return (False, key[1], woff)
        return (True, cmap[key], woff)

    plans = []
    for (k, base, q_key, qw, p_key, pw, path, scal, gamma) in raw:
        prefer_w = path in ("fact", "linear")
        q_src = resolve(q_key, qw, prefer_w)
        p_src = resolve(p_key, pw, False)
        b2_dve = B2_DVE_WHEN_EVEN and path == "fact" and pw % 2 == 0
        plans.append((k, base, q_src, p_src, path, scal, gamma, b2_dve))

    order = sorted(
        range(K), key=lambda k: (plans[k][1], plans[k][2][0] or plans[k][3][0], k)
    )  # by base, no-shift kernels first within each base run
    gcol = np.zeros((H, K), np.float32)
    for pos, k in enumerate(order):
        gcol[:, pos] = plans[k][6]
    return plans, (runlist, ncols), order, gcol


def _build(pairs_a, pairs_b, weights):
    import concourse.bacc as bacc
    import concourse.mybir as mybir
    from concourse.tile import TileContext

    f32 = mybir.dt.float32
    bf16 = mybir.dt.bfloat16
    Copy = mybir.ActivationFunctionType.Copy
    add, mult = mybir.AluOpType.add, mybir.AluOpType.mult

    plans, (runlist, ncols), order, gcol_np = _plan(pairs_a, pairs_b, weights)
    ncols = max(1, ncols)
    ngrp = (K + GRP - 1) // GRP

    if ncols > 110:
        raise RuntimeError(f"shifted-column budget exceeded: {ncols}")

    nc = bacc.Bacc()
    # flat free dim with 2 pad elements so +1-shifted runs stay in bounds
    x = nc.dram_tensor("x", [H + 4, C * FW + 2], bf16, kind="ExternalInput")
    out = nc.dram_tensor("out", [OH, K, BPC, W], bf16, kind="ExternalOutput")

    with TileContext(nc) as tc:
        with (
            tc.tile_pool(name="xp", bufs=1) as xp,
            tc.tile_pool(name="bp", bufs=10) as bp,
            tc.tile_pool(name="tp", bufs=4) as tp,
        ):
            xr = x
            # +2 element pad: flat w-offset views overread <=2 elements past
            # the last column; the pad keeps that read in-bounds.
            X = xp.tile([H, C * FW + 2], bf16)
            nc.gpsimd.memset(X[:, C * FW : C * FW + 2], 0.0)
            quarter = C * FW // 4
            for q in range(4):
                nc.sync.dma_start(
                    out=X[:, q * quarter : (q + 1) * quarter],
                    in_=xr[2 : 2 + H, q * quarter : (q + 1) * quarter],
                )

            S = xp.tile([H, ncols * FW + 2], bf16)
            nc.gpsimd.memset(S[:, ncols * FW : ncols * FW + 2], 0.0)
            for (s, sw, c0, cl, d0) in runlist:
                # S[p, d0+i] = x[c0+i, p+s] shifted sw elements left in w;
                # contiguous n-KB run per partition, full 128 partitions.
                n = cl - c0 + 1
                nc.sync.dma_start(
                    out=S[0:H, d0 * FW : (d0 + n) * FW],
                    in_=xr[2 + s : 2 + s + H, c0 * FW + sw : (c0 + n) * FW + sw],
                )

            outv = out.rearrange("oh k b w -> oh (k b w)")

            def col(src, cnt):
                shifted, idx, woff = src
                t = S if shifted else X
                return t[0:cnt, idx * FW + woff : idx * FW + woff + FW]

            gamma_ctr = [0]

            def emit_gamma_and_store(g, ks, T):
                # deferred one group so cross-engine waits are pre-satisfied
                for j, k in enumerate(ks):
                    _, base, _, _, path, scal, gamma, _ = plans[k]
                    cnt = base + OH
                    slot = T[0:cnt, j * FW : (j + 1) * FW]
                    if gamma != 0.0:
                        gamma_ctr[0] += 1
                        if gamma_ctr[0] % 2 < GAMMA_DVE_RATIO:
                            nc.vector.tensor_scalar(slot, slot, gamma, None, add)
                        else:
                            nc.scalar.activation(
                                slot, slot, Copy, bias=gamma, scale=1.0
                            )
                # batched stores per same-base run: one contiguous-run DMA per
                # run via SWDGE on the GpSimd queue (async transfer).
                i = 0
                while i < len(ks):
                    base = plans[ks[i]][1]
                    i2 = i
                    while i2 < len(ks) and plans[ks[i2]][1] == base:
                        i2 += 1
                    src = T[base : base + OH, i * FW : i2 * FW]
                    dst = outv[0:OH, (g * GRP + i) * FW : (g * GRP + i2) * FW]
                    nc.gpsimd.dma_start(out=dst, in_=src)
                    i = i2

            pending = None
            for g in range(ngrp):
                ks = order[g * GRP : (g + 1) * GRP]
                T = tp.tile([H, GRP * FW], bf16, tag="t", name=f"t_{g}")

                for j, k in enumerate(ks):
                    _, base, q_src, p_src, path, scal, gamma, b2_dve = plans[k]
                    cnt = base + OH
                    Qv, Pv = col(q_src, cnt), col(p_src, cnt)
                    slot = T[0:cnt, j * FW : (j + 1) * FW]
                    b2 = bp.tile([H, FW], bf16, tag="b2", name=f"b2_{k}")
                    b2v = b2[0:cnt]

                    if path == "fact":
                        kab, c_p, alpha = scal
                        if b2_dve:
                            nc.vector.tensor_scalar(b2v, Pv, kab, c_p, mult, add)
                        else:
                            nc.scalar.activation(b2v, Pv, Copy, bias=c_p, scale=kab)
                        nc.vector.scalar_tensor_tensor(slot, Qv, alpha, b2v, add, mult)
                    else:  # linear/exact: slot = Ca*Q + (Cb*P + C1)
                        if path == "linear":
                            kka, kkb, kk1 = scal
                        else:
                            kab, kka, kkb, kk1 = scal
                        nc.scalar.activation(b2v, Pv, Copy, bias=kk1, scale=kkb)
                        nc.vector.scalar_tensor_tensor(slot, Qv, kka, b2v, mult, add)
                        if path == "exact":  # += (Cab*P)*Q
                            p2 = bp.tile([H, FW], bf16, tag="b2", name=f"p2_{k}")
                            p2v = p2[0:cnt]
                            nc.vector.scalar_tensor_tensor(p2v, Pv, kab, Qv, mult, mult)
                            nc.vector.tensor_tensor(slot, slot, p2v, add)

                if pending is not None:
                    emit_gamma_and_store(*pending)
                pending = (g, ks, T)
            if pending is not None:
                emit_gamma_and_store(*pending)
    nc.compile()
    return nc


def _make(x, pairs_a, pairs_b, weights):
    """Build program + per-core input maps + unshard fn (shared with test)."""
    import ml_dtypes

    x = np.ascontiguousarray(np.asarray(x), dtype=np.float32)
    pa = np.asarray(pairs_a).astype(np.int64)
    pb = np.asarray(pairs_b).astype(np.int64)
    w = np.asarray(weights).astype(np.float32)

    nc = _build(pa, pb, w)
    _, _, order, _ = _plan(pa, pb, w)

    def xshard(i):
        # [BPC, C, H, W] -> [H+4, C*BPC*W + 2] with 2 zero pad rows each end
        xt = x[i * BPC : (i + 1) * BPC].transpose(2, 1, 0, 3)  # [H, C, BPC, W]
        xp = np.zeros((H + 4, C * FW + 2), ml_dtypes.bfloat16)
        xp[2 : 2 + H, : C * FW] = xt.reshape(H, C * FW).astype(ml_dtypes.bfloat16)
        return xp

    in_maps = [{"x": xshard(i)} for i in range(NCORES)]

    pos = np.empty(K, np.int64)
    pos[np.asarray(order)] = np.arange(K)

    def unshard(results):
        # device layout [OH, K(sorted), BPC, W] per core -> [B, K, OH, OW]
        full = np.concatenate(
            [r["out"] for r in results], axis=2
        )  # [OH, K, B, W]
        return np.ascontiguousarray(
            full[:, pos, :, :OW].transpose(2, 1, 0, 3).astype(np.float32)
        )

    return nc, in_maps, unshard


def kernel(x, pairs_a, pairs_b, weights):
    from concourse.bass_utils import run_bass_kernel_spmd

    nc, in_maps, unshard = _make(x, pairs_a, pairs_b, weights)
    res = run_bass_kernel_spmd(nc, in_maps, core_ids=list(range(NCORES)))
    return unshard(res.results)


# revision 11
# speedup vs baseline: 3.6608x; 1.1630x over previous
"""Trainium2 Bass kernel for nn_LogicConvSparseMatrix.

Math: the reference's 15-term weighted logic-op sum collapses to

    out[b,k] = C_ab[k]*A*B + C_a[k]*A + C_b[k]*B + C_1[k]

where A = x[b, ca_k, ha_k+oh, wa_k+ow], B = x[b, cb_k, hb_k+oh, wb_k+ow]
are shifted 126x126 windows.  Per kernel, with P = one operand and Q =
the other (orientation chosen per kernel), this factors into

    out = (Q + alpha) * (C_ab*P + c_p) + gamma

computed in bf16 as three flat element passes over full-W columns:
  1. affine:   b2 = C_ab*colP[wp:wp+FW] + c_p   (DVE 4x tensor_scalar when
               wp is even, else alignment-immune ACT)
  2. STT:      T  = (colQ[wq:wq+FW] + alpha) * b2  (DVE, 2x packed mode)
  3. + gamma   in place on T (DVE 4x tensor_scalar / ACT copy-bias split;
               never GpSimd - its SBUF ops grab the DVE shared port pair
               and stall the DVE packed modes)

Flat full-W columns: every operand is a contiguous FW = BPC*W element
slice of an SBUF column, the per-kernel w-window offset absorbed into
the slice start.  Positions w in [OW, W) per batch item are junk lanes
(the <=2-element overread past a column lands in them / in the 2-element
tile pad); the host slices w < OW after the full-W store.  DVE packed
perf modes need 4-byte-aligned bf16 operand starts, so odd w-offsets are
handled by (a) routing the affine pass to ACT for odd wp and (b) reading
the STT input from a +1-element-preshifted copy of its column.

All relative h-shifts and +1 w-shifts between the windows are resolved
HOST-side into a packed auxiliary DRAM tensor `scol` [H, ncols*FW]
holding exactly the shifted columns used, so on-device loads are two
tensors' worth of full-128-partition, contiguous-per-partition DMAs
(large descriptors spread evenly over the 16 SDMA engines; non-128-row
or small-chunk loads measurably skew/bloat descriptor work).  x channels
are host-permuted into first-use order so early compute groups unblock
after the first load chunk; kernels needing only unshifted columns are
ordered first so compute overlaps the scol load.  Device output layout
[OH, K(sorted), BPC, W] makes each group store one contiguous ~4KB run
per partition; stores issue from the GpSimd queue via SWDGE (async).
Sharding: data-parallel over batch, 2 batch items per core, 8 cores.
"""

import numpy as np

B, C, H, W = 16, 64, 128, 128
K = 128
RH = RW = 3
OH, OW = H - RH + 1, W - RW + 1
NCORES = 8
BPC = B // NCORES
FW = BPC * W  # flat column width (elements per partition per column)

GRP = 8  # kernels per store group
B2_DVE_WHEN_EVEN = True  # pass-1 affine on DVE (4x TS) when its offset is even
GAMMA_DVE_RATIO = 1  # of every 2 gamma ops, this many on DVE (rest ACT)
NXCHUNK = 4  # x load chunks
NSCHUNK = 2  # scol load chunks


def _coeffs(weights):
    """Per-kernel coefficients of out = Cab*a*b + Ca*a + Cb*b + C1."""
    w = [weights[:, i].astype(np.float64) for i in range(16)]
    cab = w[1] - w[2] - w[4] - 2 * w[6] - w[7] + w[8] + 2 * w[9] + w[11] + w[13] - w[14]
    ca = w[2] + w[3] + w[6] + w[7] - w[8] - w[9] - w[12] - w[13]
    cb = w[4] + w[5] + w[6] + w[7] - w[8] - w[9] - w[10] - w[11]
    c1 = w[8] + w[9] + w[10] + w[11] + w[12] + w[13] + w[14] + w[15]
    return cab, ca, cb, c1


def _plan(pairs_a, pairs_b, weights):
    """Host-side schedule.

    Returns (plans, layout, order) where plans[k] = (k, base, q_src, p_src,
    path, scal, gamma, b2_dve) with q_src/p_src = (from_scol, column_index,
    w_off); layout = (xperm, scolkeys): xperm = channel order in the device
    x tensor, scolkeys = [(hshift, wshift, chan)] in device scol order;
    order = group-schedulable kernel order."""
    cab, ca, cb, c1 = _coeffs(weights)
    raw = []
    for k in range(K):
        ha, wa, cca = int(pairs_a[k][0]), int(pairs_a[k][1]), int(pairs_a[k][2])
        hb, wb, ccb = int(pairs_b[k][0]), int(pairs_b[k][1]), int(pairs_b[k][2])
        # base row: window row oh lives at partition base+oh; operand side
        # with the larger h needs no h-shift when base = its h.  Choose
        # base = min(ha, hb) so the OTHER side's shift is negative... both
        # work; keep base = hb (P side unshifted) where possible after
        # orientation, else shift is resolved host-side anyway.
        kab, kka, kkb, kk1 = float(cab[k]), float(ca[k]), float(cb[k]), float(c1[k])
        cand = []
        if abs(kab) > 1e-7 and abs(kka * kkb) <= 50.0 * abs(kab):
            if abs(kkb) <= 50.0 * abs(kab):
                cand.append(("B", abs(kkb / kab)))  # P=B, Q=A
            if abs(kka) <= 50.0 * abs(kab):
                cand.append(("A", abs(kka / kab)))  # P=A, Q=B
        if cand:
            # prefer even STT-side offset (DVE 2x); tie-break smaller |alpha|
            def rank(c):
                qoff = wa if c[0] == "B" else wb
                return (qoff % 2, c[1])

            cand.sort(key=rank)
            pside = cand[0][0]
            path = "fact"
            if pside == "B":
                scal = (kab, kka, kkb / kab)
                qh, qw, qc, ph, pw, pc = ha, wa, cca, hb, wb, ccb
            else:
                scal = (kab, kkb, kka / kab)
                qh, qw, qc, ph, pw, pc = hb, wb, ccb, ha, wa, cca
            gamma = kk1 - kka * kkb / kab
        elif abs(kab) <= 1e-7:
            path, scal, gamma = "linear", (kka, kkb, kk1), 0.0
            qh, qw, qc, ph, pw, pc = ha, wa, cca, hb, wb, ccb
        else:
            path, scal, gamma = "exact", (kab, kka, kkb, kk1), 0.0
            qh, qw, qc, ph, pw, pc = ha, wa, cca, hb, wb, ccb
        # q gets a +1 w-preshifted copy when its offset is odd (STT align)
        qsw = qw % 2 if path in ("fact", "linear") else 0
        raw.append(
            (k, qh, qw, qc, qsw, ph, pw, pc, path, scal, gamma)
        )

    # column keys: base = max of the two h's (shifts then non-positive and
    # junk rows stay in the pad range).  key = (hshift, wshift, chan);
    # hshift = h - base <= 0.
    used = {}
    info = []
    for (k, qh, qw, qc, qsw, ph, pw, pc, path, scal, gamma) in raw:
        base = max(qh, ph)
        qkey = (qh - base, qsw, qc)
        pkey = (ph - base, 0, pc)
        for key in (qkey, pkey):
            if key[0] != 0 or key[1] != 0:
                used.setdefault(key, len(used))
        info.append((k, base, qkey, qw - qsw, pkey, pw, path, scal, gamma))

    # schedule order: kernels with both columns unshifted first (they only
    # need the x tensor), then the rest; base-sorted inside for store runs.
    def needs_scol(i):
        _, _, qkey, _, pkey, _, _, _, _ = info[i]
        return (qkey[0], qkey[1]) != (0, 0) or (pkey[0], pkey[1]) != (0, 0)

    order = sorted(range(K), key=lambda i: (needs_scol(i), info[i][1], i))

    # x channel permutation: first-use order over the schedule
    xperm = []
    seen = set()
    for i in order:
        _, _, qkey, _, pkey, _, _, _, _ = info[i]
        for key in (qkey, pkey):
            if key[0] == 0 and key[1] == 0 and key[2] not in seen:
                seen.add(key[2])
                xperm.append(key[2])
    for c in range(C):
        if c not in seen:
            xperm.append(c)
    xpos = {c: j for j, c in enumerate(xperm)}

    # scol keys in first-use order over the schedule
    sperm = []
    sseen = set()
    for i in order:
        _, _, qkey, _, pkey, _, _, _, _ = info[i]
        for key in (qkey, pkey):
            if (key[0] != 0 or key[1] != 0) and key not in sseen:
                sseen.add(key)
                sperm.append(key)
    spos = {key: j for j, key in enumerate(sperm)}

    plans = [None] * K
    for (k, base, qkey, qoff, pkey, poff, path, scal, gamma) in info:
        def src(key, off):
            if key[0] == 0 and key[1] == 0:
                return (False, xpos[key[2]], off)
            return (True, spos[key], off)

        b2_dve = B2_DVE_WHEN_EVEN and path == "fact" and poff % 2 == 0
        plans[k] = (k, base, src(qkey, qoff), src(pkey, poff), path, scal,
                    gamma, b2_dve)

    return plans, (xperm, sperm), order


def _build(pairs_a, pairs_b, weights):
    import concourse.bacc as bacc
    import concourse.mybir as mybir
    from concourse.tile import TileContext

    bf16 = mybir.dt.bfloat16
    Copy = mybir.ActivationFunctionType.Copy
    add, mult = mybir.AluOpType.add, mybir.AluOpType.mult

    plans, (xperm, sperm), order = _plan(pairs_a, pairs_b, weights)
    ncols = max(1, len(sperm))
    ngrp = (K + GRP - 1) // GRP

    nc = bacc.Bacc()
    x = nc.dram_tensor("x", [H, C * FW], bf16, kind="ExternalInput")
    sc = nc.dram_tensor("scol", [H, ncols * FW], bf16, kind="ExternalInput")
    out = nc.dram_tensor("out", [OH, K, BPC, W], bf16, kind="ExternalOutput")

    with TileContext(nc) as tc:
        with (
            tc.tile_pool(name="xp", bufs=1) as xp,
            tc.tile_pool(name="bp", bufs=10) as bp,
            tc.tile_pool(name="tp", bufs=4) as tp,
        ):
            # +2 element pad: flat w-offset views overread <=2 elements past
            # the last column; the pad keeps that read in-bounds.
            X = xp.tile([H, C * FW + 2], bf16)
            nc.gpsimd.memset(X[:, C * FW : C * FW + 2], 0.0)
            S = xp.tile([H, ncols * FW + 2], bf16)
            nc.gpsimd.memset(S[:, ncols * FW : ncols * FW + 2], 0.0)

            bnd = [C * FW * q // NXCHUNK for q in range(NXCHUNK + 1)]
            for q in range(NXCHUNK):
                nc.sync.dma_start(
                    out=X[:, bnd[q] : bnd[q + 1]], in_=x[:, bnd[q] : bnd[q + 1]]
                )
            sbnd = [ncols * FW * q // NSCHUNK for q in range(NSCHUNK + 1)]
            for q in range(NSCHUNK):
                nc.sync.dma_start(
                    out=S[:, sbnd[q] : sbnd[q + 1]],
                    in_=sc[:, sbnd[q] : sbnd[q + 1]],
                )

            outv = out.rearrange("oh k b w -> oh (k b w)")

            def col(src, cnt):
                shifted, idx, woff = src
                t = S if shifted else X
                return t[0:cnt, idx * FW + woff : idx * FW + woff + FW]

            gamma_ctr = [0]

            def emit_gamma_and_store(g, ks, T):
                # deferred one group so cross-engine waits are pre-satisfied
                for j, k in enumerate(ks):
                    _, base, _, _, path, scal, gamma, _ = plans[k]
                    cnt = base + OH
                    slot = T[0:cnt, j * FW : (j + 1) * FW]
                    if gamma != 0.0:
                        gamma_ctr[0] += 1
                        if gamma_ctr[0] % 2 < GAMMA_DVE_RATIO:
                            nc.vector.tensor_scalar(slot, slot, gamma, None, add)
                        else:
                            nc.scalar.activation(
                                slot, slot, Copy, bias=gamma, scale=1.0
                            )
                # batched stores per same-base run: one contiguous-run DMA per
                # run via SWDGE on the GpSimd queue (async transfer).
                i = 0
                while i < len(ks):
                    base = plans[ks[i]][1]
                    i2 = i
                    while i2 < len(ks) and plans[ks[i2]][1] == base:
                        i2 += 1
                    src = T[base : base + OH, i * FW : i2 * FW]
                    dst = outv[0:OH, (g * GRP + i) * FW : (g * GRP + i2) * FW]
                    nc.gpsimd.dma_start(out=dst, in_=src)
                    i = i2

            pending = None
            for g in range(ngrp):
                ks = order[g * GRP : (g + 1) * GRP]
                T = tp.tile([H, GRP * FW], bf16, tag="t", name=f"t_{g}")

                for j, k in enumerate(ks):
                    _, base, q_src, p_src, path, scal, gamma, b2_dve = plans[k]
                    cnt = base + OH
                    Qv, Pv = col(q_src, cnt), col(p_src, cnt)
                    slot = T[0:cnt, j * FW : (j + 1) * FW]
                    b2 = bp.tile([H, FW], bf16, tag="b2", name=f"b2_{k}")
                    b2v = b2[0:cnt]

                    if path == "fact":
                        kab, c_p, alpha = scal
                        if b2_dve:
                            nc.vector.tensor_scalar(b2v, Pv, kab, c_p, mult, add)
                        else:
                            nc.scalar.activation(b2v, Pv, Copy, bias=c_p, scale=kab)
                        nc.vector.scalar_tensor_tensor(slot, Qv, alpha, b2v, add, mult)
                    else:  # linear/exact: slot = Ca*Q + (Cb*P + C1)
                        if path == "linear":
                            kka, kkb, kk1 = scal
                        else:
                            kab, kka, kkb, kk1 = scal
                        nc.scalar.activation(b2v, Pv, Copy, bias=kk1, scale=kkb)
                        nc.vector.scalar_tensor_tensor(slot, Qv, kka, b2v, mult, add)
                        if path == "exact":  # += (Cab*P)*Q
                            p2 = bp.tile([H, FW], bf16, tag="b2", name=f"p2_{k}")
                            p2v = p2[0:cnt]
                            nc.vector.scalar_tensor_tensor(p2v, Pv, kab, Qv, mult, mult)
                            nc.vector.tensor_tensor(slot, slot, p2v, add)

                if pending is not None:
                    emit_gamma_and_store(*pending)
                pending = (g, ks, T)
            if pending is not None:
                emit_gamma_and_store(*pending)
    nc.compile()
    return nc


def _make(x, pairs_a, pairs_b, weights):
    """Build program + per-core input maps + unshard fn (shared with test)."""
    import ml_dtypes

    x = np.ascontiguousarray(np.asarray(x), dtype=np.float32)
    pa = np.asarray(pairs_a).astype(np.int64)
    pb = np.asarray(pairs_b).astype(np.int64)
    w = np.asarray(weights).astype(np.float32)

    nc = _build(pa, pb, w)
    plans, (xperm, sperm), order = _plan(pa, pb, w)
    ncols = max(1, len(sperm))

    def shards(i):
        # [BPC, C, H, W] -> [H, C(perm), BPC, W] flat, bf16
        xt = (
            x[i * BPC : (i + 1) * BPC]
            .transpose(2, 1, 0, 3)
            .astype(ml_dtypes.bfloat16)
        )  # [H, C, BPC, W]
        xflat = np.ascontiguousarray(xt[:, xperm]).reshape(H, C * FW)
        # scol: host-resolved shifted columns, zero-filled out of range
        scol = np.zeros((H, ncols, FW), ml_dtypes.bfloat16)
        flat = xt.reshape(H, C * FW)
        for j, (sh, sw, c) in enumerate(sperm):
            # scol[p, j, f] = x[p+sh, c*FW + f + sw]
            lo, hi = max(0, -sh), min(H, H - sh)
            seg = flat[lo + sh : hi + sh, c * FW + sw : (c + 1) * FW + sw]
            if seg.shape[1] < FW:  # w-shift ran past the tensor end
                pad = np.zeros((seg.shape[0], FW - seg.shape[1]), ml_dtypes.bfloat16)
                seg = np.concatenate([seg, pad], axis=1)
            scol[lo:hi, j] = seg
        return {"x": xflat, "scol": scol.reshape(H, ncols * FW)}

    in_maps = [shards(i) for i in range(NCORES)]

    pos = np.empty(K, np.int64)
    pos[np.asarray(order)] = np.arange(K)

    def unshard(results):
        # device layout [OH, K(sorted), BPC, W] per core -> [B, K, OH, OW]
        full = np.concatenate(
            [r["out"] for r in results], axis=2
        )  # [OH, K, B, W]
        return np.ascontiguousarray(
            full[:, pos, :, :OW].transpose(2, 1, 0, 3).astype(np.float32)
        )

    return nc, in_maps, unshard


def kernel(x, pairs_a, pairs_b, weights):
    from concourse.bass_utils import run_bass_kernel_spmd

    nc, in_maps, unshard = _make(x, pairs_a, pairs_b, weights)
    res = run_bass_kernel_spmd(nc, in_maps, core_ids=list(range(NCORES)))
    return unshard(res.results)


# revision 17
# speedup vs baseline: 3.7391x; 1.0214x over previous
"""Trainium2 Bass kernel for nn_LogicConvSparseMatrix.

Math: the reference's 15-term weighted logic-op sum collapses to

    out[b,k] = C_ab[k]*A*B + C_a[k]*A + C_b[k]*B + C_1[k]

where A = x[b, ca_k, ha_k+oh, wa_k+ow], B = x[b, cb_k, hb_k+oh, wb_k+ow]
are shifted 126x126 windows.  Per kernel, with P = one operand and Q =
the other (orientation chosen per kernel), this factors into

    out = (Q + alpha) * (C_ab*P + c_p) + gamma

computed in bf16 as three flat element passes over full-W columns:
  1. affine:   b2 = C_ab*colP[wp:wp+FW] + c_p   (DVE 4x tensor_scalar when
               wp is even, else alignment-immune ACT)
  2. STT:      T  = (colQ[wq:wq+FW] + alpha) * b2  (DVE, 2x packed mode)
  3. + gamma   in place on T (DVE 4x tensor_scalar / ACT copy-bias split;
               never GpSimd - its SBUF ops grab the DVE shared port pair
               and stall the DVE packed modes)

Flat full-W columns: every operand is a contiguous FW = BPC*W element
slice of an SBUF column, the per-kernel w-window offset absorbed into
the slice start.  Positions w in [OW, W) per batch item are junk lanes
(the <=2-element overread past a column lands in them / in the 2-element
tile pad); the host slices w < OW after the full-W store.  DVE packed
perf modes need 4-byte-aligned bf16 operand starts, so odd w-offsets are
handled by (a) routing the affine pass to ACT for odd wp and (b) reading
the STT input from a +1-element-preshifted copy of its column.

All relative h-shifts and +1 w-shifts between the windows are resolved
HOST-side into a packed auxiliary DRAM tensor `scol` [H, ncols*FW]
holding exactly the shifted columns used, so on-device loads are two
tensors' worth of full-128-partition, contiguous-per-partition DMAs
(large descriptors spread evenly over the 16 SDMA engines; non-128-row
or small-chunk loads measurably skew/bloat descriptor work).  x channels
are host-permuted into first-use order so early compute groups unblock
after the first load chunk; kernels needing only unshifted columns are
ordered first so compute overlaps the scol load.  Device output layout
[OH, K(sorted), BPC, W] makes each group store one contiguous ~4KB run
per partition; stores issue from the GpSimd queue via SWDGE (async).
Sharding: data-parallel over batch, 2 batch items per core, 8 cores.
"""

import numpy as np

B, C, H, W = 16, 64, 128, 128
K = 128
RH = RW = 3
OH, OW = H - RH + 1, W - RW + 1
NCORES = 8
BPC = B // NCORES
FW = BPC * W  # flat column width (elements per partition per column)

GRP = 8  # kernels per store group
B2_DVE_WHEN_EVEN = True  # pass-1 affine on DVE (4x TS) when its offset is even
GAMMA_DVE_MOD, GAMMA_DVE_LT = 7, 3  # gamma ops with ctr%MOD<LT go to DVE


def _coeffs(weights):
    """Per-kernel coefficients of out = Cab*a*b + Ca*a + Cb*b + C1."""
    w = [weights[:, i].astype(np.float64) for i in range(16)]
    cab = w[1] - w[2] - w[4] - 2 * w[6] - w[7] + w[8] + 2 * w[9] + w[11] + w[13] - w[14]
    ca = w[2] + w[3] + w[6] + w[7] - w[8] - w[9] - w[12] - w[13]
    cb = w[4] + w[5] + w[6] + w[7] - w[8] - w[9] - w[10] - w[11]
    c1 = w[8] + w[9] + w[10] + w[11] + w[12] + w[13] + w[14] + w[15]
    return cab, ca, cb, c1


def _plan(pairs_a, pairs_b, weights):
    """Host-side schedule.

    Returns (plans, layout, order) where plans[k] = (k, base, q_src, p_src,
    path, scal, gamma, b2_dve) with q_src/p_src = (from_scol, column_index,
    w_off); layout = (xperm, scolkeys): xperm = channel order in the device
    x tensor, scolkeys = [(hshift, wshift, chan)] in device scol order;
    order = group-schedulable kernel order."""
    cab, ca, cb, c1 = _coeffs(weights)
    raw = []
    for k in range(K):
        ha, wa, cca = int(pairs_a[k][0]), int(pairs_a[k][1]), int(pairs_a[k][2])
        hb, wb, ccb = int(pairs_b[k][0]), int(pairs_b[k][1]), int(pairs_b[k][2])
        # base row: window row oh lives at partition base+oh; operand side
        # with the larger h needs no h-shift when base = its h.  Choose
        # base = min(ha, hb) so the OTHER side's shift is negative... both
        # work; keep base = hb (P side unshifted) where possible after
        # orientation, else shift is resolved host-side anyway.
        kab, kka, kkb, kk1 = float(cab[k]), float(ca[k]), float(cb[k]), float(c1[k])
        cand = []
        if abs(kab) > 1e-7 and abs(kka * kkb) <= 50.0 * abs(kab):
            if abs(kkb) <= 50.0 * abs(kab):
                cand.append(("B", abs(kkb / kab)))  # P=B, Q=A
            if abs(kka) <= 50.0 * abs(kab):
                cand.append(("A", abs(kka / kab)))  # P=A, Q=B
        if cand:
            # prefer even STT-side offset (DVE 2x); tie-break smaller |alpha|
            def rank(c):
                qoff = wa if c[0] == "B" else wb
                return (qoff % 2, c[1])

            cand.sort(key=rank)
            pside = cand[0][0]
            path = "fact"
            if pside == "B":
                scal = (kab, kka, kkb / kab)
                qh, qw, qc, ph, pw, pc = ha, wa, cca, hb, wb, ccb
            else:
                scal = (kab, kkb, kka / kab)
                qh, qw, qc, ph, pw, pc = hb, wb, ccb, ha, wa, cca
            gamma = kk1 - kka * kkb / kab
        elif abs(kab) <= 1e-7:
            path, scal, gamma = "linear", (kka, kkb, kk1), 0.0
            qh, qw, qc, ph, pw, pc = ha, wa, cca, hb, wb, ccb
        else:
            path, scal, gamma = "exact", (kab, kka, kkb, kk1), 0.0
            qh, qw, qc, ph, pw, pc = ha, wa, cca, hb, wb, ccb
        # q gets a +1 w-preshifted copy when its offset is odd (STT align)
        qsw = qw % 2 if path in ("fact", "linear") else 0
        raw.append(
            (k, qh, qw, qc, qsw, ph, pw, pc, path, scal, gamma)
        )

    # column keys: base = max of the two h's (shifts then non-positive and
    # junk rows stay in the pad range).  key = (hshift, wshift, chan);
    # hshift = h - base <= 0.
    used = {}
    info = []
    for (k, qh, qw, qc, qsw, ph, pw, pc, path, scal, gamma) in raw:
        base = max(qh, ph)
        qkey = (qh - base, qsw, qc)
        pkey = (ph - base, 0, pc)
        for key in (qkey, pkey):
            if key[0] != 0 or key[1] != 0:
                used.setdefault(key, len(used))
        info.append((k, base, qkey, qw - qsw, pkey, pw, path, scal, gamma))

    # schedule order: kernels with both columns unshifted first (they only
    # need the x tensor), then the rest; base-sorted inside for store runs.
    def needs_scol(i):
        _, _, qkey, _, pkey, _, _, _, _ = info[i]
        return (qkey[0], qkey[1]) != (0, 0) or (pkey[0], pkey[1]) != (0, 0)

    order = sorted(range(K), key=lambda i: (needs_scol(i), info[i][1], i))

    # x channel permutation: first-use order over the schedule
    xperm = []
    seen = set()
    for i in order:
        _, _, qkey, _, pkey, _, _, _, _ = info[i]
        for key in (qkey, pkey):
            if key[0] == 0 and key[1] == 0 and key[2] not in seen:
                seen.add(key[2])
                xperm.append(key[2])
    for c in range(C):
        if c not in seen:
            xperm.append(c)
    xpos = {c: j for j, c in enumerate(xperm)}

    # scol keys in first-use order over the schedule
    sperm = []
    sseen = set()
    for i in order:
        _, _, qkey, _, pkey, _, _, _, _ = info[i]
        for key in (qkey, pkey):
            if (key[0] != 0 or key[1] != 0) and key not in sseen:
                sseen.add(key)
                sperm.append(key)
    spos = {key: j for j, key in enumerate(sperm)}

    plans = [None] * K
    for (k, base, qkey, qoff, pkey, poff, path, scal, gamma) in info:
        def src(key, off):
            if key[0] == 0 and key[1] == 0:
                return (False, xpos[key[2]], off)
            return (True, spos[key], off)

        b2_dve = B2_DVE_WHEN_EVEN and path == "fact" and poff % 2 == 0
        plans[k] = (k, base, src(qkey, qoff), src(pkey, poff), path, scal,
                    gamma, b2_dve)

    return plans, (xperm, sperm), order


def _build(pairs_a, pairs_b, weights):
    import concourse.bacc as bacc
    import concourse.mybir as mybir
    from concourse.tile import TileContext

    bf16 = mybir.dt.bfloat16
    Copy = mybir.ActivationFunctionType.Copy
    add, mult = mybir.AluOpType.add, mybir.AluOpType.mult

    plans, (xperm, sperm), order = _plan(pairs_a, pairs_b, weights)
    ncols = max(1, len(sperm))
    ngrp = (K + GRP - 1) // GRP

    nc = bacc.Bacc()
    x = nc.dram_tensor("x", [H, C * FW], bf16, kind="ExternalInput")
    sc = nc.dram_tensor("scol", [H, ncols * FW], bf16, kind="ExternalInput")
    out = nc.dram_tensor("out", [OH, K, BPC, W], bf16, kind="ExternalOutput")

    with TileContext(nc) as tc:
        with (
            tc.tile_pool(name="xp", bufs=1) as xp,
            tc.tile_pool(name="bp", bufs=10) as bp,
            tc.tile_pool(name="tp", bufs=4) as tp,
        ):
            # +2 element pad: flat w-offset views overread <=2 elements past
            # the last column; the pad keeps that read in-bounds.
            X = xp.tile([H, C * FW + 2], bf16)
            nc.gpsimd.memset(X[:, C * FW : C * FW + 2], 0.0)
            S = xp.tile([H, ncols * FW + 2], bf16)
            nc.gpsimd.memset(S[:, ncols * FW : ncols * FW + 2], 0.0)

            # interleaved issue order: a small first x chunk unblocks the
            # first compute groups early; scol chunks slot between x chunks
            # (channels are first-use ordered, scol consumers come later).
            xbnd = [0, 8 * FW, 24 * FW, 44 * FW, C * FW]
            sbnd = [0, ncols * FW // 2, ncols * FW]
            seq = [("x", 0), ("x", 1), ("s", 0), ("x", 2), ("s", 1), ("x", 3)]
            for kind, q in seq:
                if kind == "x":
                    nc.sync.dma_start(
                        out=X[:, xbnd[q] : xbnd[q + 1]],
                        in_=x[:, xbnd[q] : xbnd[q + 1]],
                    )
                else:
                    nc.sync.dma_start(
                        out=S[:, sbnd[q] : sbnd[q + 1]],
                        in_=sc[:, sbnd[q] : sbnd[q + 1]],
                    )

            outv = out.rearrange("oh k b w -> oh (k b w)")

            def col(src, cnt):
                shifted, idx, woff = src
                t = S if shifted else X
                return t[0:cnt, idx * FW + woff : idx * FW + woff + FW]

            gamma_ctr = [0]

            def emit_gamma(j, k, T):
                _, base, _, _, path, scal, gamma, _ = plans[k]
                cnt = base + OH
                slot = T[0:cnt, j * FW : (j + 1) * FW]
                if gamma != 0.0:
                    gamma_ctr[0] += 1
                    if gamma_ctr[0] % GAMMA_DVE_MOD < GAMMA_DVE_LT:
                        nc.vector.tensor_scalar(slot, slot, gamma, None, add)
                    else:
                        nc.scalar.activation(slot, slot, Copy, bias=gamma, scale=1.0)

            def emit_store(g, ks, T):
                # batched stores per same-base run: one contiguous-run DMA per
                # run via SWDGE on the GpSimd queue (async transfer).
                i = 0
                while i < len(ks):
                    base = plans[ks[i]][1]
                    i2 = i
                    while i2 < len(ks) and plans[ks[i2]][1] == base:
                        i2 += 1
                    src = T[base : base + OH, i * FW : i2 * FW]
                    dst = outv[0:OH, (g * GRP + i) * FW : (g * GRP + i2) * FW]
                    nc.gpsimd.dma_start(out=dst, in_=src)
                    i = i2

            def emit_gamma_and_store(g, ks, T):
                # deferred one group so cross-engine waits are pre-satisfied
                for j, k in enumerate(ks):
                    emit_gamma(j, k, T)
                emit_store(g, ks, T)

            pending = None
            for g in range(ngrp):
                ks = order[g * GRP : (g + 1) * GRP]
                last = g == ngrp - 1
                T = tp.tile([H, GRP * FW], bf16, tag="t", name=f"t_{g}")

                for j, k in enumerate(ks):
                    _, base, q_src, p_src, path, scal, gamma, b2_dve = plans[k]
                    cnt = base + OH
                    Qv, Pv = col(q_src, cnt), col(p_src, cnt)
                    slot = T[0:cnt, j * FW : (j + 1) * FW]
                    b2 = bp.tile([H, FW], bf16, tag="b2", name=f"b2_{k}")
                    b2v = b2[0:cnt]

                    if path == "fact":
                        kab, c_p, alpha = scal
                        if b2_dve:
                            nc.vector.tensor_scalar(b2v, Pv, kab, c_p, mult, add)
                        else:
                            nc.scalar.activation(b2v, Pv, Copy, bias=c_p, scale=kab)
                        nc.vector.scalar_tensor_tensor(slot, Qv, alpha, b2v, add, mult)
                    else:  # linear/exact: slot = Ca*Q + (Cb*P + C1)
                        if path == "linear":
                            kka, kkb, kk1 = scal
                        else:
                            kab, kka, kkb, kk1 = scal
                        nc.scalar.activation(b2v, Pv, Copy, bias=kk1, scale=kkb)
                        nc.vector.scalar_tensor_tensor(slot, Qv, kka, b2v, mult, add)
                        if path == "exact":  # += (Cab*P)*Q
                            p2 = bp.tile([H, FW], bf16, tag="b2", name=f"p2_{k}")
                            p2v = p2[0:cnt]
                            nc.vector.scalar_tensor_tensor(p2v, Pv, kab, Qv, mult, mult)
                            nc.vector.tensor_tensor(slot, slot, p2v, add)
                    if last:
                        # no deferral on the final group: shortens the tail
                        emit_gamma(j, k, T)

                if pending is not None:
                    emit_gamma_and_store(*pending)
                pending = (g, ks, T) if not last else ("store", g, ks, T)
            if pending is not None:
                if pending[0] == "store":
                    emit_store(*pending[1:])
                else:
                    emit_gamma_and_store(*pending)
    nc.compile()
    return nc


def _make(x, pairs_a, pairs_b, weights):
    """Build program + per-core input maps + unshard fn (shared with test)."""
    import ml_dtypes

    x = np.ascontiguousarray(np.asarray(x), dtype=np.float32)
    pa = np.asarray(pairs_a).astype(np.int64)
    pb = np.asarray(pairs_b).astype(np.int64)
    w = np.asarray(weights).astype(np.float32)

    nc = _build(pa, pb, w)
    plans, (xperm, sperm), order = _plan(pa, pb, w)
    ncols = max(1, len(sperm))

    def shards(i):
        # [BPC, C, H, W] -> [H, C(perm), BPC, W] flat, bf16
        xt = (
            x[i * BPC : (i + 1) * BPC]
            .transpose(2, 1, 0, 3)
            .astype(ml_dtypes.bfloat16)
        )  # [H, C, BPC, W]
        xflat = np.ascontiguousarray(xt[:, xperm]).reshape(H, C * FW)
        # scol: host-resolved shifted columns, zero-filled out of range
        scol = np.zeros((H, ncols, FW), ml_dtypes.bfloat16)
        flat = xt.reshape(H, C * FW)
        for j, (sh, sw, c) in enumerate(sperm):
            # scol[p, j, f] = x[p+sh, c*FW + f + sw]
            lo, hi = max(0, -sh), min(H, H - sh)
            seg = flat[lo + sh : hi + sh, c * FW + sw : (c + 1) * FW + sw]
            if seg.shape[1] < FW:  # w-shift ran past the tensor end
                pad = np.zeros((seg.shape[0], FW - seg.shape[1]), ml_dtypes.bfloat16)
                seg = np.concatenate([seg, pad], axis=1)
            scol[lo:hi, j] = seg
        return {"x": xflat, "scol": scol.reshape(H, ncols * FW)}

    in_maps = [shards(i) for i in range(NCORES)]

    pos = np.empty(K, np.int64)
    pos[np.asarray(order)] = np.arange(K)

    def unshard(results):
        # device layout [OH, K(sorted), BPC, W] per core -> [B, K, OH, OW]
        full = np.concatenate(
            [r["out"] for r in results], axis=2
        )  # [OH, K, B, W]
        return np.ascontiguousarray(
            full[:, pos, :, :OW].transpose(2, 1, 0, 3).astype(np.float32)
        )

    return nc, in_maps, unshard


def kernel(x, pairs_a, pairs_b, weights):
    from concourse.bass_utils import run_bass_kernel_spmd

    nc, in_maps, unshard = _make(x, pairs_a, pairs_b, weights)
    res = run_bass_kernel_spmd(nc, in_maps, core_ids=list(range(NCORES)))
    return unshard(res.results)
